# revision 8
# baseline (speedup 1.0000x reference)
"""ABMIL (attention-based MIL) Trainium2 kernel, 8-core data-parallel.

Shapes (hardcoded): B=8 bags, N=1024 instances, D=1024, H=16 heads, hd=64.
Each NeuronCore processes one bag. Parameters are replicated; all big
tensors are host-packed into [128, F] layouts so every DMA is 128
contiguous descriptors (one per partition). Big DMAs all go on the sync
HWDGE ring in strict priority order (ring FIFO = bandwidth priority):
x first (longest dependent chain), then w_q, w_k, w_v, w_o.

Math (rank-1 attention: only the cls query row survives):
  znorm      = (z - mu) * rsqrt(var + eps)          rows of z = [cls; x_b]
  ln0        = znorm0 * gamma + beta
  q          = w_q @ ln0 + b_q
  U[h]       = w_k[64h:64h+64].T @ q[64h:64h+64]            (16 x 1024)
  Ug         = U * gamma * 0.125 ;  e_h = 0.125*(U[h]@beta + q_h@b_k_h)
  scores     = Ug @ znorm.T + e                              (16 x 1025)
  A          = softmax(scores)
  Mrow       = (A @ znorm) * gamma + beta                    (16 x 1024)
  ctx[64h:+64] = w_v[64h:+64] @ Mrow[h] + b_v[64h:+64]
  out        = w_o @ ctx + b_o
"""

import numpy as np
import ml_dtypes

import concourse.bass as bass
import concourse.bacc as bacc
import concourse.mybir as mybir
import concourse.tile as tile
from concourse.bass_utils import run_bass_kernel_spmd

F32 = mybir.dt.float32
BF16 = mybir.dt.bfloat16
AX = mybir.AxisListType.X
OP = mybir.AluOpType
AF = mybir.ActivationFunctionType

D = 1024
NK = 8          # number of 128-chunks of D (and of x rows)
H = 16
EPS = 1e-5

_CACHE = {}


def _build():
    nc = bacc.Bacc()

    # Host-packed inputs: every [128, F] tensor DMAs as 128 contiguous rows.
    x_e = nc.declare_dram_parameter("x", [128, NK * D], BF16, isOutput=False)
    wq_e = nc.declare_dram_parameter("wqp", [128, NK * D], BF16, isOutput=False)
    # wk pack carries b_k (column layout) in its last 8 columns
    wk_e = nc.declare_dram_parameter("wkp", [128, NK * D + NK], BF16, isOutput=False)
    wv_e = nc.declare_dram_parameter("wvp", [128, NK * D], BF16, isOutput=False)
    wo_e = nc.declare_dram_parameter("wop", [128, NK * D], BF16, isOutput=False)
    # smalls: [gam_col, bet_col, bq_col, bv_col] each [128, 8] f32
    sm_e = nc.declare_dram_parameter("smalls", [128, 4 * NK], F32, isOutput=False)
    # clsbo: row 0 = cls_token, row 1 = b_o (both f32)
    cb_e = nc.declare_dram_parameter("clsbo", [2, D], F32, isOutput=False)
    gam_e = nc.declare_dram_parameter("gam", [D], F32, isOutput=False)
    bet_e = nc.declare_dram_parameter("bet", [D], F32, isOutput=False)
    out_e = nc.declare_dram_parameter("out", [1, D], F32, isOutput=True)

    with tile.TileContext(nc) as tc:
        with (
            tc.tile_pool(name="singles", bufs=1) as singles,
            tc.tile_pool(name="xin", bufs=8) as xin,
            tc.tile_pool(name="work", bufs=4) as work,
            tc.tile_pool(name="pt", bufs=2, space="PSUM") as pt,
            tc.tile_pool(name="pbig", bufs=2, space="PSUM") as pbig,
            tc.tile_pool(name="psm", bufs=2, space="PSUM") as psm,
        ):
            ident = singles.tile([128, 128], BF16, tag="ident")
            ident_dram = nc.inline_tensor(
                np.eye(128, dtype=ml_dtypes.bfloat16), name="ident_const"
            )

            eps_t = singles.tile([128, 1], F32, tag="eps")
            nc.vector.memset(eps_t[:, :], EPS)
            one_t = singles.tile([1, 1], F32, tag="one")
            nc.vector.memset(one_t[:, :], 1.0)

            cls_row = singles.tile([1, D], F32, tag="clsr")
            bo_row = singles.tile([1, D], F32, tag="bor")
            sm = singles.tile([128, 4 * NK], F32, tag="smalls")
            gam_col = sm[:, 0 * NK : 1 * NK]
            bet_col = sm[:, 1 * NK : 2 * NK]
            bq_col = sm[:, 2 * NK : 3 * NK]
            bv_col = sm[:, 3 * NK : 4 * NK]

            # big persistent tiles
            wq_all = singles.tile([128, NK * D], BF16, tag="wq")
            wk_all = singles.tile([128, NK * D + NK], BF16, tag="wk")
            bk_col = wk_all[:, NK * D : NK * D + NK]
            wv_all = singles.tile([128, NK * D], BF16, tag="wv")
            wo_all = singles.tile([128, NK * D], BF16, tag="wo")
            znorm_all = singles.tile([128, NK * D], BF16, tag="znorm")
            znT_all = singles.tile([128, NK * D], BF16, tag="znT")
            znT3 = znT_all[:, :].rearrange("p (c s) -> p c s", c=NK)

            xks = [xin.tile([128, D], BF16, tag="xk", name=f"xk{i}") for i in range(NK)]

            # --- DMA schedule -------------------------------------------
            # scalar ring: tiny loads + the 8 znorm transposes (emitted in
            #   the x loop below) + final out store.
            # sync ring: all big HBM loads, strict priority order.
            nc.scalar.dma_start(out=cls_row[:, :], in_=cb_e[0:1, :])
            nc.scalar.dma_start(out=bo_row[:, :], in_=cb_e[1:2, :])
            nc.scalar.dma_start(out=sm[:, :], in_=sm_e[:, :])
            nc.scalar.dma_start(out=ident[:, :], in_=ident_dram[:, :])
            for k in range(4):
                nc.sync.dma_start(out=xks[k][:, :], in_=x_e[:, D * k : D * (k + 1)])
            nc.sync.dma_start(out=wq_all[:, :], in_=wq_e[:, :])
            for k in range(4, NK):
                nc.sync.dma_start(out=xks[k][:, :], in_=x_e[:, D * k : D * (k + 1)])
            nc.sync.dma_start(out=wk_all[:, :], in_=wk_e[:, :])
            nc.sync.dma_start(out=wv_all[:, :], in_=wv_e[:, :])
            nc.sync.dma_start(out=wo_all[:, :], in_=wo_e[:, :])

            gam16 = singles.tile([H, D], F32, tag="gam16")
            nc.gpsimd.dma_start(
                out=gam16[:, :],
                in_=bass.AP(tensor=gam_e[:].tensor, offset=0, ap=[[0, H], [1, D]]),
            )
            bet16 = singles.tile([H, D], F32, tag="bet16")
            nc.gpsimd.dma_start(
                out=bet16[:, :],
                in_=bass.AP(tensor=bet_e[:].tensor, offset=0, ap=[[0, H], [1, D]]),
            )

            # ---- ACT table pre-warm (load Sqrt/Exp/Copy LUTs at t~0) ----
            warm = work.tile([1, 1], F32, tag="warm")
            nc.scalar.activation(out=warm[:, :], in_=eps_t[0:1, :], func=AF.Sqrt,
                                 bias=0.0, scale=1.0)
            nc.scalar.activation(out=warm[:, :], in_=eps_t[0:1, :], func=AF.Exp,
                                 bias=0.0, scale=1.0)
            nc.scalar.activation(out=warm[:, :], in_=eps_t[0:1, :], func=AF.Copy,
                                 bias=0.0, scale=1.0)

            # ---- cls row LN --------------------------------------------
            stats0 = work.tile([1, 2, 6], F32, tag="stats0")
            nc.vector.bn_stats(out=stats0[:, 0, :], in_=cls_row[:, 0:512])
            nc.vector.bn_stats(out=stats0[:, 1, :], in_=cls_row[:, 512:1024])
            mv0 = work.tile([1, 2], F32, tag="mv0")
            nc.vector.bn_aggr(out=mv0[:, :], in_=stats0[:, :, :])
            nc.scalar.activation(
                out=mv0[:, 1:2], in_=mv0[:, 1:2], func=AF.Sqrt,
                bias=eps_t[0:1, :], scale=1.0,
            )
            rs0 = work.tile([1, 1], F32, tag="rs0")
            nc.vector.tensor_copy(out=rs0[:, :], in_=mv0[:, 1:2])
            nc.vector.reciprocal(out=rs0[:, :], in_=rs0[:, :])
            zn0_row = singles.tile([1, D], BF16, tag="zn0r")
            nc.vector.tensor_scalar(
                out=zn0_row[:, :], in0=cls_row[:, :],
                scalar1=mv0[:, 0:1], scalar2=rs0[:, 0:1],
                op0=OP.subtract, op1=OP.mult,
            )
            lzp = pt.tile([128, 16], BF16, tag="pt")
            for c in range(NK):
                nc.tensor.transpose(
                    out=lzp[:, 2 * c : 2 * c + 1],
                    in_=zn0_row[0:1, 128 * c : 128 * (c + 1)],
                    identity=ident[0:1, 0:1],
                )
            zn0_col = singles.tile([128, NK], BF16, tag="zn0c")
            nc.scalar.copy(
                out=zn0_col[:, :],
                in_=lzp[:, :].rearrange("p (c x) -> p c x", c=NK)[:, :, 0],
            )
            ln0_col = singles.tile([128, NK], BF16, tag="ln0c")
            nc.vector.tensor_mul(out=ln0_col[:, :], in0=zn0_col[:, :], in1=gam_col[:, :])
            nc.vector.tensor_add(out=ln0_col[:, :], in0=ln0_col[:, :], in1=bet_col[:, :])

            # ---- x LayerNorm pipeline (emitted before weight-dependent
            #      ops so the DVE/ACT streams never block on weights) -----
            for k in range(NK):
                xk = xks[k]
                stats = work.tile([128, 2, 6], F32, tag="stats", name=f"stats{k}")
                nc.vector.bn_stats(out=stats[:, 0, :], in_=xk[:, 0:512])
                nc.vector.bn_stats(out=stats[:, 1, :], in_=xk[:, 512:1024])
                mv = work.tile([128, 2], F32, tag="mv", name=f"mv{k}")
                nc.vector.bn_aggr(out=mv[:, :], in_=stats[:, :, :])
                nc.scalar.activation(
                    out=mv[:, 1:2], in_=mv[:, 1:2], func=AF.Sqrt,
                    bias=eps_t[:, :], scale=1.0,
                )
                rs = work.tile([128, 1], F32, tag="rs", name=f"rs{k}")
                nc.vector.tensor_copy(out=rs[:, :], in_=mv[:, 1:2])
                nc.vector.reciprocal(out=rs[:, :], in_=rs[:, :])
                nc.vector.tensor_scalar(
                    out=znorm_all[:, D * k : D * (k + 1)], in0=xk[:, :],
                    scalar1=mv[:, 0:1], scalar2=rs[:, 0:1],
                    op0=OP.subtract, op1=OP.mult,
                )
                nc.scalar.dma_start_transpose(
                    out=znT3[:, :, 128 * k : 128 * (k + 1)],
                    in_=znorm_all[:, D * k : D * (k + 1)],
                )

            # ---- q = ln0 @ w_q.T + b_q ---------------------------------
            psq = pbig.tile([1, D], F32, tag="pbig")
            for c in range(NK):
                for half in range(2):
                    nc.tensor.matmul(
                        psq[:, 512 * half : 512 * (half + 1)], lhsT=ln0_col[:, c : c + 1],
                        rhs=wq_all[:, D * c + 512 * half : D * c + 512 * (half + 1)],
                        start=(c == 0), stop=(c == NK - 1),
                        skip_group_check=True,
                    )
            q_sb = singles.tile([1, D], BF16, tag="qsb")
            nc.scalar.copy(out=q_sb[:, :], in_=psq[:, :])
            qcp = pt.tile([128, 16], BF16, tag="pt")
            for c in range(NK):
                nc.tensor.transpose(
                    out=qcp[:, 2 * c : 2 * c + 1],
                    in_=q_sb[0:1, 128 * c : 128 * (c + 1)],
                    identity=ident[0:1, 0:1],
                )
            q_col = singles.tile([128, NK], BF16, tag="qcol")
            nc.scalar.copy(
                out=q_col[:, :],
                in_=qcp[:, :].rearrange("p (c x) -> p c x", c=NK)[:, :, 0],
            )
            nc.vector.tensor_add(out=q_col[:, :], in0=q_col[:, :], in1=bq_col[:, :])
            qbT = singles.tile([128, H * NK], BF16, tag="qbT")
            nc.gpsimd.memset(qbT[:, :], 0.0)
            for c in range(NK):
                nc.gpsimd.tensor_copy(
                    out=qbT[0:64, H * c + 2 * c : H * c + 2 * c + 1],
                    in_=q_col[0:64, c : c + 1],
                )
                nc.gpsimd.tensor_copy(
                    out=qbT[64:128, H * c + 2 * c + 1 : H * c + 2 * c + 2],
                    in_=q_col[64:128, c : c + 1],
                )

            # ---- U = Qblk @ w_k ; Ug, e, softmax shift -----------------
            psU = pbig.tile([H, D], F32, tag="pbig")
            for c in range(NK):
                for half in range(2):
                    nc.tensor.matmul(
                        psU[:, 512 * half : 512 * (half + 1)], lhsT=qbT[:, H * c : H * (c + 1)],
                        rhs=wk_all[:, D * c + 512 * half : D * c + 512 * (half + 1)],
                        start=(c == 0), stop=(c == NK - 1),
                        skip_group_check=True,
                    )
            pse2 = psm.tile([H, 1], F32, tag="psm")
            for c in range(NK):
                nc.tensor.matmul(
                    pse2[:, :], lhsT=qbT[:, H * c : H * (c + 1)], rhs=bk_col[:, c : c + 1],
                    start=(c == 0), stop=(c == NK - 1),
                )
            ug = singles.tile([H, D], BF16, tag="ug")
            nc.vector.scalar_tensor_tensor(
                out=ug[:, :], in0=psU[:, :], scalar=0.125, in1=gam16[:, :],
                op0=OP.mult, op1=OP.mult,
            )
            tmp16 = work.tile([H, D], F32, tag="tmp16")
            nc.vector.tensor_mul(out=tmp16[:, :], in0=psU[:, :], in1=bet16[:, :])
            e1 = work.tile([H, 1], F32, tag="e1")
            nc.vector.reduce_sum(out=e1[:, :], in_=tmp16[:, :], axis=AX)
            e_sb = singles.tile([H, 1], F32, tag="esb")
            nc.vector.tensor_add(out=e_sb[:, :], in0=e1[:, :], in1=pse2[:, :])
            nc.vector.tensor_scalar_mul(out=e_sb[:, :], in0=e_sb[:, :], scalar1=0.125)

            ugp = pt.tile([128, 128], BF16, tag="pt")
            for c in range(NK):
                nc.tensor.transpose(
                    out=ugp[:, H * c : H * (c + 1)], in_=ug[:, 128 * c : 128 * (c + 1)],
                    identity=ident[0:H, 0:H],
                )
            ugT = singles.tile([128, H * NK], BF16, tag="ugT")
            nc.scalar.copy(out=ugT[:, :], in_=ugp[:, :])

            # per-head safe softmax shift: bound_h = 8*||Ug_h|| >= max score
            u2 = work.tile([H, D], F32, tag="u2")
            nc.vector.tensor_mul(out=u2[:, :], in0=ug[:, :], in1=ug[:, :])
            s2 = work.tile([H, 1], F32, tag="s2")
            nc.vector.reduce_sum(out=s2[:, :], in_=u2[:, :], axis=AX)
            bound = work.tile([H, 1], F32, tag="bound")
            nc.scalar.activation(
                out=bound[:, :], in_=s2[:, :], func=AF.Sqrt, bias=0.0, scale=64.0
            )
            eb = work.tile([H, 1], F32, tag="eb")
            nc.vector.tensor_sub(out=eb[:, :], in0=e_sb[:, :], in1=bound[:, :])

            # ---- cls score / attention prologue ------------------------
            a_sb = singles.tile([H, 1025], BF16, tag="asb")
            aT = singles.tile([128, H * NK], BF16, tag="aT")
            se = work.tile([H, 3], F32, tag="seall")
            se0 = se[:, 2:3]
            psS0 = psm.tile([H, 1], F32, tag="psm")
            for c in range(NK):
                nc.tensor.matmul(
                    psS0[:, :], lhsT=ugT[:, H * c : H * (c + 1)], rhs=zn0_col[:, c : c + 1],
                    start=(c == 0), stop=(c == NK - 1),
                )
            nc.scalar.activation(
                out=a_sb[:, 0:1], in_=psS0[:, :], func=AF.Exp,
                bias=eb[:, 0:1], scale=1.0, accum_out=se0,
            )
            a0p = pt.tile([128, 16], BF16, tag="pt")
            nc.tensor.transpose(out=a0p[0:1, 0:H], in_=a_sb[:, 0:1], identity=ident[0:H, 0:H])
            aT0 = singles.tile([1, H], BF16, tag="aT0")
            nc.scalar.copy(out=aT0[:, :], in_=a0p[0:1, 0:H])
            psM = pbig.tile([H, D], F32, tag="pbig")
            for half in range(2):
                nc.tensor.matmul(
                    psM[:, 512 * half : 512 * (half + 1)], lhsT=aT0[:, :],
                    rhs=zn0_row[:, 512 * half : 512 * (half + 1)],
                    start=True, stop=False, skip_group_check=True,
                )

            # ---- scores/softmax per 512-col half, then Mrow ------------
            for half in range(2):
                psS = psm.tile([H, 512], F32, tag="psm", name=f"psS{half}")
                for c in range(NK):
                    nc.tensor.matmul(
                        psS[:, :], lhsT=ugT[:, H * c : H * (c + 1)],
                        rhs=znT_all[:, D * c + 512 * half : D * c + 512 * (half + 1)],
                        start=(c == 0), stop=(c == NK - 1),
                    )
                nc.scalar.activation(
                    out=a_sb[:, 1 + 512 * half : 1 + 512 * (half + 1)], in_=psS[:, :],
                    func=AF.Exp, bias=eb[:, 0:1], scale=1.0,
                    accum_out=se[:, half : half + 1],
                )
                for kk in range(4):
                    k = 4 * half + kk
                    atpk = pt.tile([128, 16], BF16, tag="pt", name=f"atp{k}")
                    nc.tensor.transpose(
                        out=atpk[:, 0:H],
                        in_=a_sb[:, 1 + 128 * k : 1 + 128 * (k + 1)],
                        identity=ident[0:H, 0:H],
                    )
                    nc.scalar.copy(out=aT[:, H * k : H * (k + 1)], in_=atpk[:, 0:H])
                    for dh in range(2):
                        nc.tensor.matmul(
                            psM[:, 512 * dh : 512 * (dh + 1)], lhsT=aT[:, H * k : H * (k + 1)],
                            rhs=znorm_all[:, D * k + 512 * dh : D * k + 512 * (dh + 1)],
                            start=False, stop=(k == NK - 1), skip_group_check=True,
                        )

            rinv = work.tile([H, 1], F32, tag="rinv")
            nc.vector.reduce_sum(out=rinv[:, :], in_=se[:, :], axis=AX)
            nc.vector.reciprocal(out=rinv[:, :], in_=rinv[:, :])

            # mrow = psM * rinv * gamma  (beta added after transpose)
            mrow = singles.tile([H, D], BF16, tag="mrow")
            nc.vector.scalar_tensor_tensor(
                out=mrow[:, :], in0=psM[:, :], scalar=rinv[:, 0:1], in1=gam16[:, :],
                op0=OP.mult, op1=OP.mult,
            )
            mT = singles.tile([128, H * NK], BF16, tag="mT")
            for c in range(NK):
                mtp = pt.tile([128, 16], BF16, tag="pt", name=f"mtp{c}")
                nc.tensor.transpose(
                    out=mtp[:, 0:H], in_=mrow[:, 128 * c : 128 * (c + 1)],
                    identity=ident[0:H, 0:H],
                )
                # fold the PSUM->SBUF copy into the beta add
                nc.vector.tensor_scalar_add(
                    out=mT[:, H * c : H * (c + 1)], in0=mtp[:, 0:H],
                    scalar1=bet_col[:, c : c + 1],
                )

            # ---- ctx via [16,512] trick + transpose-select --------------
            # ps16[h, n] = sum_d Mln[h, d] * w_v[n, d]; ctx[n] = ps16[n//64, n]
            ps16 = pbig.tile([H, D], F32, tag="pbig")
            for half in range(2):
                for c in range(NK):
                    nc.tensor.matmul(
                        ps16[:, 512 * half : 512 * (half + 1)], lhsT=mT[:, H * c : H * (c + 1)],
                        rhs=wv_all[:, D * c + 512 * half : D * c + 512 * (half + 1)],
                        start=(c == 0), stop=(c == NK - 1),
                        skip_group_check=True,
                    )
            c16 = singles.tile([H, D], BF16, tag="c16")
            nc.scalar.copy(out=c16[:, :], in_=ps16[:, :])
            ctx_bf = singles.tile([128, NK], BF16, tag="ctxbf")
            psO = pbig.tile([1, D], F32, tag="pbig")
            for j in range(NK):
                ctp = pt.tile([128, 16], BF16, tag="pt", name=f"ctp{j}")
                nc.tensor.transpose(
                    out=ctp[:, 0:H], in_=c16[:, 128 * j : 128 * (j + 1)],
                    identity=ident[0:H, 0:H],
                )
                nc.vector.scalar_tensor_tensor(
                    out=ctx_bf[0:64, j : j + 1], in0=ctp[0:64, 2 * j : 2 * j + 1],
                    scalar=1.0, in1=bv_col[0:64, j : j + 1], op0=OP.mult, op1=OP.add,
                )
                nc.vector.scalar_tensor_tensor(
                    out=ctx_bf[64:128, j : j + 1], in0=ctp[64:128, 2 * j + 1 : 2 * j + 2],
                    scalar=1.0, in1=bv_col[64:128, j : j + 1], op0=OP.mult, op1=OP.add,
                )
            for a in range(NK):
                for half in range(2):
                    nc.tensor.matmul(
                        psO[:, 512 * half : 512 * (half + 1)],
                        lhsT=ctx_bf[:, a : a + 1],
                        rhs=wo_all[:, D * a + 512 * half : D * a + 512 * (half + 1)],
                        start=(a == 0), stop=False,
                        skip_group_check=True,
                    )
            for half in range(2):
                nc.tensor.matmul(
                    psO[:, 512 * half : 512 * (half + 1)], lhsT=one_t[:, :],
                    rhs=bo_row[0:1, 512 * half : 512 * (half + 1)],
                    start=False, stop=True, skip_group_check=True,
                )
            out_sb = singles.tile([1, D], F32, tag="outsb")
            nc.scalar.copy(out=out_sb[:, :], in_=psO[:, :])
            nc.scalar.dma_start(out=out_e[:, :], in_=out_sb[:, :])

    nc.compile()
    return nc


def _pack128(a):
    # [1024, F] -> [128, 8*F] with out[p, k*F+i] = a[128k+p, i]
    rows, cols = a.shape
    return np.ascontiguousarray(
        a.reshape(NK, 128, cols).transpose(1, 0, 2).reshape(128, NK * cols)
    )


def _col(a):
    # [1024] -> [128, 8] with out[p, c] = a[128c+p]
    return np.ascontiguousarray(a.reshape(NK, 128).T)


def _prep_in_maps(inputs):
    bf = ml_dtypes.bfloat16
    f32 = np.float32

    def c(a, dt):
        return np.asarray(a, dtype=dt)

    x = c(inputs["x"], bf)
    smalls = np.concatenate(
        [
            _col(c(inputs["gamma"], f32)),
            _col(c(inputs["beta"], f32)),
            _col(c(inputs["b_q"], f32)),
            _col(c(inputs["b_v"], f32)),
        ],
        axis=1,
    )
    clsbo = np.stack([c(inputs["cls_token"], f32), c(inputs["b_o"], f32)])
    wkp = np.concatenate([_pack128(c(inputs["w_k"], bf)), _col(c(inputs["b_k"], bf))], axis=1)
    shared = {
        "gam": np.ascontiguousarray(c(inputs["gamma"], f32)),
        "bet": np.ascontiguousarray(c(inputs["beta"], f32)),
        "smalls": np.ascontiguousarray(smalls),
        "clsbo": np.ascontiguousarray(clsbo),
        "wqp": _pack128(c(np.asarray(inputs["w_q"]).T, bf)),
        "wkp": np.ascontiguousarray(wkp),
        "wvp": _pack128(c(np.asarray(inputs["w_v"]).T, bf)),
        "wop": _pack128(c(np.asarray(inputs["w_o"]).T, bf)),
    }
    return [{"x": _pack128(x[b]), **shared} for b in range(8)]


def run(inputs, trace=False, **kw):
    if "nc" not in _CACHE:
        _CACHE["nc"] = _build()
    nc = _CACHE["nc"]
    in_maps = _prep_in_maps(inputs)
    res = run_bass_kernel_spmd(nc, in_maps, core_ids=list(range(8)), trace=trace, **kw)
    out = np.stack([np.asarray(res.results[b]["out"][0], dtype=np.float32) for b in range(8)])
    return out, res


def kernel(**inputs):
    out, _ = run(inputs, trace=False)
    return out


# revision 9
# speedup vs baseline: 1.0003x; 1.0003x over previous
"""ABMIL (attention-based MIL) Trainium2 kernel, 8-core data-parallel.

Shapes (hardcoded): B=8 bags, N=1024 instances, D=1024, H=16 heads, hd=64.
Each NeuronCore processes one bag. Parameters are replicated; all big
tensors are host-packed into [128, F] layouts so every DMA is 128
contiguous descriptors (one per partition). Big DMAs all go on the sync
HWDGE ring in strict priority order (ring FIFO = bandwidth priority):
x first (longest dependent chain), then w_q, w_k, w_v, w_o.

Math (rank-1 attention: only the cls query row survives):
  znorm      = (z - mu) * rsqrt(var + eps)          rows of z = [cls; x_b]
  ln0        = znorm0 * gamma + beta
  q          = w_q @ ln0 + b_q
  U[h]       = w_k[64h:64h+64].T @ q[64h:64h+64]            (16 x 1024)
  Ug         = U * gamma * 0.125 ;  e_h = 0.125*(U[h]@beta + q_h@b_k_h)
  scores     = Ug @ znorm.T + e                              (16 x 1025)
  A          = softmax(scores)
  Mrow       = (A @ znorm) * gamma + beta                    (16 x 1024)
  ctx[64h:+64] = w_v[64h:+64] @ Mrow[h] + b_v[64h:+64]
  out        = w_o @ ctx + b_o
"""

import numpy as np
import ml_dtypes

import concourse.bass as bass
import concourse.bacc as bacc
import concourse.mybir as mybir
import concourse.tile as tile
from concourse.bass_utils import run_bass_kernel_spmd

F32 = mybir.dt.float32
BF16 = mybir.dt.bfloat16
AX = mybir.AxisListType.X
OP = mybir.AluOpType
AF = mybir.ActivationFunctionType

D = 1024
NK = 8          # number of 128-chunks of D (and of x rows)
H = 16
EPS = 1e-5

_CACHE = {}


def _build():
    nc = bacc.Bacc()

    # Host-packed inputs: every [128, F] tensor DMAs as 128 contiguous rows.
    x_e = nc.declare_dram_parameter("x", [128, NK * D], BF16, isOutput=False)
    wq_e = nc.declare_dram_parameter("wqp", [128, NK * D], BF16, isOutput=False)
    # wk pack carries b_k (column layout) in its last 8 columns
    wk_e = nc.declare_dram_parameter("wkp", [128, NK * D + NK], BF16, isOutput=False)
    wv_e = nc.declare_dram_parameter("wvp", [128, NK * D], BF16, isOutput=False)
    wo_e = nc.declare_dram_parameter("wop", [128, NK * D], BF16, isOutput=False)
    # smalls: [gam_col, bet_col, bq_col, bv_col] each [128, 8] f32
    sm_e = nc.declare_dram_parameter("smalls", [128, 4 * NK], F32, isOutput=False)
    # clsbo: row 0 = cls_token, row 1 = b_o (both f32)
    cb_e = nc.declare_dram_parameter("clsbo", [2, D], F32, isOutput=False)
    gam_e = nc.declare_dram_parameter("gam", [D], F32, isOutput=False)
    bet_e = nc.declare_dram_parameter("bet", [D], F32, isOutput=False)
    out_e = nc.declare_dram_parameter("out", [1, D], F32, isOutput=True)

    with tile.TileContext(nc) as tc:
        with (
            tc.tile_pool(name="singles", bufs=1) as singles,
            tc.tile_pool(name="xin", bufs=8) as xin,
            tc.tile_pool(name="work", bufs=4) as work,
            tc.tile_pool(name="pt", bufs=2, space="PSUM") as pt,
            tc.tile_pool(name="pbig", bufs=2, space="PSUM") as pbig,
            tc.tile_pool(name="psm", bufs=2, space="PSUM") as psm,
        ):
            ident = singles.tile([128, 128], BF16, tag="ident")
            ident_dram = nc.inline_tensor(
                np.eye(128, dtype=ml_dtypes.bfloat16), name="ident_const"
            )

            eps_t = singles.tile([128, 1], F32, tag="eps")
            nc.vector.memset(eps_t[:, :], EPS)
            one_t = singles.tile([1, 1], F32, tag="one")
            nc.vector.memset(one_t[:, :], 1.0)

            cls_row = singles.tile([1, D], F32, tag="clsr")
            bo_row = singles.tile([1, D], F32, tag="bor")
            sm = singles.tile([128, 4 * NK], F32, tag="smalls")
            gam_col = sm[:, 0 * NK : 1 * NK]
            bet_col = sm[:, 1 * NK : 2 * NK]
            bq_col = sm[:, 2 * NK : 3 * NK]
            bv_col = sm[:, 3 * NK : 4 * NK]

            # big persistent tiles
            wq_all = singles.tile([128, NK * D], BF16, tag="wq")
            wk_all = singles.tile([128, NK * D + NK], BF16, tag="wk")
            bk_col = wk_all[:, NK * D : NK * D + NK]
            wv_all = singles.tile([128, NK * D], BF16, tag="wv")
            wo_all = singles.tile([128, NK * D], BF16, tag="wo")
            znorm_all = singles.tile([128, NK * D], BF16, tag="znorm")
            znT_all = singles.tile([128, NK * D], BF16, tag="znT")
            znT3 = znT_all[:, :].rearrange("p (c s) -> p c s", c=NK)

            xall = singles.tile([128, NK * D], BF16, tag="xall")
            xks = [xall[:, D * i : D * (i + 1)] for i in range(NK)]

            # --- DMA schedule -------------------------------------------
            # scalar ring: tiny loads + the 8 znorm transposes (emitted in
            #   the x loop below) + final out store.
            # sync ring: all big HBM loads, strict priority order, 8KB+
            #   per-partition descriptors (one queue saturates HBM).
            nc.scalar.dma_start(out=cls_row[:, :], in_=cb_e[0:1, :])
            nc.scalar.dma_start(out=bo_row[:, :], in_=cb_e[1:2, :])
            nc.scalar.dma_start(out=sm[:, :], in_=sm_e[:, :])
            nc.scalar.dma_start(out=ident[:, :], in_=ident_dram[:, :])
            nc.sync.dma_start(out=xall[:, 0 : 4 * D], in_=x_e[:, 0 : 4 * D])
            nc.sync.dma_start(out=xall[:, 4 * D : 8 * D], in_=x_e[:, 4 * D : 8 * D])
            nc.sync.dma_start(out=wq_all[:, :], in_=wq_e[:, :])
            nc.sync.dma_start(out=wk_all[:, :], in_=wk_e[:, :])
            nc.sync.dma_start(out=wv_all[:, :], in_=wv_e[:, :])
            nc.sync.dma_start(out=wo_all[:, :], in_=wo_e[:, :])

            gam16 = singles.tile([H, D], F32, tag="gam16")
            nc.gpsimd.dma_start(
                out=gam16[:, :],
                in_=bass.AP(tensor=gam_e[:].tensor, offset=0, ap=[[0, H], [1, D]]),
            )
            bet16 = singles.tile([H, D], F32, tag="bet16")
            nc.gpsimd.dma_start(
                out=bet16[:, :],
                in_=bass.AP(tensor=bet_e[:].tensor, offset=0, ap=[[0, H], [1, D]]),
            )

            # ---- ACT table pre-warm (load Sqrt/Exp/Copy LUTs at t~0) ----
            warm = work.tile([1, 1], F32, tag="warm")
            nc.scalar.activation(out=warm[:, :], in_=eps_t[0:1, :], func=AF.Sqrt,
                                 bias=0.0, scale=1.0)
            nc.scalar.activation(out=warm[:, :], in_=eps_t[0:1, :], func=AF.Exp,
                                 bias=0.0, scale=1.0)
            nc.scalar.activation(out=warm[:, :], in_=eps_t[0:1, :], func=AF.Copy,
                                 bias=0.0, scale=1.0)

            # ---- cls row LN --------------------------------------------
            stats0 = work.tile([1, 2, 6], F32, tag="stats0")
            nc.vector.bn_stats(out=stats0[:, 0, :], in_=cls_row[:, 0:512])
            nc.vector.bn_stats(out=stats0[:, 1, :], in_=cls_row[:, 512:1024])
            mv0 = work.tile([1, 2], F32, tag="mv0")
            nc.vector.bn_aggr(out=mv0[:, :], in_=stats0[:, :, :])
            nc.scalar.activation(
                out=mv0[:, 1:2], in_=mv0[:, 1:2], func=AF.Sqrt,
                bias=eps_t[0:1, :], scale=1.0,
            )
            rs0 = work.tile([1, 1], F32, tag="rs0")
            nc.vector.tensor_copy(out=rs0[:, :], in_=mv0[:, 1:2])
            nc.vector.reciprocal(out=rs0[:, :], in_=rs0[:, :])
            zn0_row = singles.tile([1, D], BF16, tag="zn0r")
            nc.vector.tensor_scalar(
                out=zn0_row[:, :], in0=cls_row[:, :],
                scalar1=mv0[:, 0:1], scalar2=rs0[:, 0:1],
                op0=OP.subtract, op1=OP.mult,
            )
            lzp = pt.tile([128, 16], BF16, tag="pt")
            for c in range(NK):
                nc.tensor.transpose(
                    out=lzp[:, 2 * c : 2 * c + 1],
                    in_=zn0_row[0:1, 128 * c : 128 * (c + 1)],
                    identity=ident[0:1, 0:1],
                )
            zn0_col = singles.tile([128, NK], BF16, tag="zn0c")
            nc.scalar.copy(
                out=zn0_col[:, :],
                in_=lzp[:, :].rearrange("p (c x) -> p c x", c=NK)[:, :, 0],
            )
            ln0_col = singles.tile([128, NK], BF16, tag="ln0c")
            nc.vector.tensor_mul(out=ln0_col[:, :], in0=zn0_col[:, :], in1=gam_col[:, :])
            nc.vector.tensor_add(out=ln0_col[:, :], in0=ln0_col[:, :], in1=bet_col[:, :])

            # ---- x LayerNorm pipeline (emitted before weight-dependent
            #      ops so the DVE/ACT streams never block on weights) -----
            for k in range(NK):
                xk = xks[k]
                stats = work.tile([128, 2, 6], F32, tag="stats", name=f"stats{k}")
                nc.vector.bn_stats(out=stats[:, 0, :], in_=xk[:, 0:512])
                nc.vector.bn_stats(out=stats[:, 1, :], in_=xk[:, 512:1024])
                mv = work.tile([128, 2], F32, tag="mv", name=f"mv{k}")
                nc.vector.bn_aggr(out=mv[:, :], in_=stats[:, :, :])
                nc.scalar.activation(
                    out=mv[:, 1:2], in_=mv[:, 1:2], func=AF.Sqrt,
                    bias=eps_t[:, :], scale=1.0,
                )
                rs = work.tile([128, 1], F32, tag="rs", name=f"rs{k}")
                nc.vector.tensor_copy(out=rs[:, :], in_=mv[:, 1:2])
                nc.vector.reciprocal(out=rs[:, :], in_=rs[:, :])
                nc.vector.tensor_scalar(
                    out=znorm_all[:, D * k : D * (k + 1)], in0=xk[:, :],
                    scalar1=mv[:, 0:1], scalar2=rs[:, 0:1],
                    op0=OP.subtract, op1=OP.mult,
                )
                nc.scalar.dma_start_transpose(
                    out=znT3[:, :, 128 * k : 128 * (k + 1)],
                    in_=znorm_all[:, D * k : D * (k + 1)],
                )

            # ---- q = ln0 @ w_q.T + b_q ---------------------------------
            psq = pbig.tile([1, D], F32, tag="pbig")
            for c in range(NK):
                for half in range(2):
                    nc.tensor.matmul(
                        psq[:, 512 * half : 512 * (half + 1)], lhsT=ln0_col[:, c : c + 1],
                        rhs=wq_all[:, D * c + 512 * half : D * c + 512 * (half + 1)],
                        start=(c == 0), stop=(c == NK - 1),
                        skip_group_check=True,
                    )
            q_sb = singles.tile([1, D], BF16, tag="qsb")
            nc.scalar.copy(out=q_sb[:, :], in_=psq[:, :])
            qcp = pt.tile([128, 16], BF16, tag="pt")
            for c in range(NK):
                nc.tensor.transpose(
                    out=qcp[:, 2 * c : 2 * c + 1],
                    in_=q_sb[0:1, 128 * c : 128 * (c + 1)],
                    identity=ident[0:1, 0:1],
                )
            q_col = singles.tile([128, NK], BF16, tag="qcol")
            nc.scalar.copy(
                out=q_col[:, :],
                in_=qcp[:, :].rearrange("p (c x) -> p c x", c=NK)[:, :, 0],
            )
            nc.vector.tensor_add(out=q_col[:, :], in0=q_col[:, :], in1=bq_col[:, :])
            qbT = singles.tile([128, H * NK], BF16, tag="qbT")
            nc.gpsimd.memset(qbT[:, :], 0.0)
            for c in range(NK):
                nc.gpsimd.tensor_copy(
                    out=qbT[0:64, H * c + 2 * c : H * c + 2 * c + 1],
                    in_=q_col[0:64, c : c + 1],
                )
                nc.gpsimd.tensor_copy(
                    out=qbT[64:128, H * c + 2 * c + 1 : H * c + 2 * c + 2],
                    in_=q_col[64:128, c : c + 1],
                )

            # ---- U = Qblk @ w_k ; Ug, e, softmax shift -----------------
            psU = pbig.tile([H, D], F32, tag="pbig")
            for c in range(NK):
                for half in range(2):
                    nc.tensor.matmul(
                        psU[:, 512 * half : 512 * (half + 1)], lhsT=qbT[:, H * c : H * (c + 1)],
                        rhs=wk_all[:, D * c + 512 * half : D * c + 512 * (half + 1)],
                        start=(c == 0), stop=(c == NK - 1),
                        skip_group_check=True,
                    )
            pse2 = psm.tile([H, 1], F32, tag="psm")
            for c in range(NK):
                nc.tensor.matmul(
                    pse2[:, :], lhsT=qbT[:, H * c : H * (c + 1)], rhs=bk_col[:, c : c + 1],
                    start=(c == 0), stop=(c == NK - 1),
                )
            ug = singles.tile([H, D], BF16, tag="ug")
            nc.vector.scalar_tensor_tensor(
                out=ug[:, :], in0=psU[:, :], scalar=0.125, in1=gam16[:, :],
                op0=OP.mult, op1=OP.mult,
            )
            tmp16 = work.tile([H, D], F32, tag="tmp16")
            nc.vector.tensor_mul(out=tmp16[:, :], in0=psU[:, :], in1=bet16[:, :])
            e1 = work.tile([H, 1], F32, tag="e1")
            nc.vector.reduce_sum(out=e1[:, :], in_=tmp16[:, :], axis=AX)
            e_sb = singles.tile([H, 1], F32, tag="esb")
            nc.vector.tensor_add(out=e_sb[:, :], in0=e1[:, :], in1=pse2[:, :])
            nc.vector.tensor_scalar_mul(out=e_sb[:, :], in0=e_sb[:, :], scalar1=0.125)

            ugp = pt.tile([128, 128], BF16, tag="pt")
            for c in range(NK):
                nc.tensor.transpose(
                    out=ugp[:, H * c : H * (c + 1)], in_=ug[:, 128 * c : 128 * (c + 1)],
                    identity=ident[0:H, 0:H],
                )
            ugT = singles.tile([128, H * NK], BF16, tag="ugT")
            nc.scalar.copy(out=ugT[:, :], in_=ugp[:, :])

            # per-head safe softmax shift: bound_h = 8*||Ug_h|| >= max score
            u2 = work.tile([H, D], F32, tag="u2")
            nc.vector.tensor_mul(out=u2[:, :], in0=ug[:, :], in1=ug[:, :])
            s2 = work.tile([H, 1], F32, tag="s2")
            nc.vector.reduce_sum(out=s2[:, :], in_=u2[:, :], axis=AX)
            bound = work.tile([H, 1], F32, tag="bound")
            nc.scalar.activation(
                out=bound[:, :], in_=s2[:, :], func=AF.Sqrt, bias=0.0, scale=64.0
            )
            eb = work.tile([H, 1], F32, tag="eb")
            nc.vector.tensor_sub(out=eb[:, :], in0=e_sb[:, :], in1=bound[:, :])

            # ---- cls score / attention prologue ------------------------
            a_sb = singles.tile([H, 1025], BF16, tag="asb")
            aT = singles.tile([128, H * NK], BF16, tag="aT")
            se = work.tile([H, 3], F32, tag="seall")
            se0 = se[:, 2:3]
            psS0 = psm.tile([H, 1], F32, tag="psm")
            for c in range(NK):
                nc.tensor.matmul(
                    psS0[:, :], lhsT=ugT[:, H * c : H * (c + 1)], rhs=zn0_col[:, c : c + 1],
                    start=(c == 0), stop=(c == NK - 1),
                )
            nc.scalar.activation(
                out=a_sb[:, 0:1], in_=psS0[:, :], func=AF.Exp,
                bias=eb[:, 0:1], scale=1.0, accum_out=se0,
            )
            a0p = pt.tile([128, 16], BF16, tag="pt")
            nc.tensor.transpose(out=a0p[0:1, 0:H], in_=a_sb[:, 0:1], identity=ident[0:H, 0:H])
            aT0 = singles.tile([1, H], BF16, tag="aT0")
            nc.scalar.copy(out=aT0[:, :], in_=a0p[0:1, 0:H])
            psM = pbig.tile([H, D], F32, tag="pbig")
            for half in range(2):
                nc.tensor.matmul(
                    psM[:, 512 * half : 512 * (half + 1)], lhsT=aT0[:, :],
                    rhs=zn0_row[:, 512 * half : 512 * (half + 1)],
                    start=True, stop=False, skip_group_check=True,
                )

            # ---- scores/softmax per 512-col half, then Mrow ------------
            for half in range(2):
                psS = psm.tile([H, 512], F32, tag="psm", name=f"psS{half}")
                for c in range(NK):
                    nc.tensor.matmul(
                        psS[:, :], lhsT=ugT[:, H * c : H * (c + 1)],
                        rhs=znT_all[:, D * c + 512 * half : D * c + 512 * (half + 1)],
                        start=(c == 0), stop=(c == NK - 1),
                    )
                nc.scalar.activation(
                    out=a_sb[:, 1 + 512 * half : 1 + 512 * (half + 1)], in_=psS[:, :],
                    func=AF.Exp, bias=eb[:, 0:1], scale=1.0,
                    accum_out=se[:, half : half + 1],
                )
                for kk in range(4):
                    k = 4 * half + kk
                    atpk = pt.tile([128, 16], BF16, tag="pt", name=f"atp{k}")
                    nc.tensor.transpose(
                        out=atpk[:, 0:H],
                        in_=a_sb[:, 1 + 128 * k : 1 + 128 * (k + 1)],
                        identity=ident[0:H, 0:H],
                    )
                    nc.scalar.copy(out=aT[:, H * k : H * (k + 1)], in_=atpk[:, 0:H])
                    for dh in range(2):
                        nc.tensor.matmul(
                            psM[:, 512 * dh : 512 * (dh + 1)], lhsT=aT[:, H * k : H * (k + 1)],
                            rhs=znorm_all[:, D * k + 512 * dh : D * k + 512 * (dh + 1)],
                            start=False, stop=(k == NK - 1), skip_group_check=True,
                        )

            rinv = work.tile([H, 1], F32, tag="rinv")
            nc.vector.reduce_sum(out=rinv[:, :], in_=se[:, :], axis=AX)
            nc.vector.reciprocal(out=rinv[:, :], in_=rinv[:, :])

            # mrow = psM * rinv * gamma  (beta added after transpose)
            mrow = singles.tile([H, D], BF16, tag="mrow")
            nc.vector.scalar_tensor_tensor(
                out=mrow[:, :], in0=psM[:, :], scalar=rinv[:, 0:1], in1=gam16[:, :],
                op0=OP.mult, op1=OP.mult,
            )
            mT = singles.tile([128, H * NK], BF16, tag="mT")
            for c in range(NK):
                mtp = pt.tile([128, 16], BF16, tag="pt", name=f"mtp{c}")
                nc.tensor.transpose(
                    out=mtp[:, 0:H], in_=mrow[:, 128 * c : 128 * (c + 1)],
                    identity=ident[0:H, 0:H],
                )
                # fold the PSUM->SBUF copy into the beta add
                nc.vector.tensor_scalar_add(
                    out=mT[:, H * c : H * (c + 1)], in0=mtp[:, 0:H],
                    scalar1=bet_col[:, c : c + 1],
                )

            # ---- ctx via [16,512] trick + transpose-select --------------
            # ps16[h, n] = sum_d Mln[h, d] * w_v[n, d]; ctx[n] = ps16[n//64, n]
            ps16 = pbig.tile([H, D], F32, tag="pbig")
            for half in range(2):
                for c in range(NK):
                    nc.tensor.matmul(
                        ps16[:, 512 * half : 512 * (half + 1)], lhsT=mT[:, H * c : H * (c + 1)],
                        rhs=wv_all[:, D * c + 512 * half : D * c + 512 * (half + 1)],
                        start=(c == 0), stop=(c == NK - 1),
                        skip_group_check=True,
                    )
            c16 = singles.tile([H, D], BF16, tag="c16")
            nc.scalar.copy(out=c16[:, :], in_=ps16[:, :])
            ctx_bf = singles.tile([128, NK], BF16, tag="ctxbf")
            psO = pbig.tile([1, D], F32, tag="pbig")
            for j in range(NK):
                ctp = pt.tile([128, 16], BF16, tag="pt", name=f"ctp{j}")
                nc.tensor.transpose(
                    out=ctp[:, 0:H], in_=c16[:, 128 * j : 128 * (j + 1)],
                    identity=ident[0:H, 0:H],
                )
                nc.vector.scalar_tensor_tensor(
                    out=ctx_bf[0:64, j : j + 1], in0=ctp[0:64, 2 * j : 2 * j + 1],
                    scalar=1.0, in1=bv_col[0:64, j : j + 1], op0=OP.mult, op1=OP.add,
                )
                nc.vector.scalar_tensor_tensor(
                    out=ctx_bf[64:128, j : j + 1], in0=ctp[64:128, 2 * j + 1 : 2 * j + 2],
                    scalar=1.0, in1=bv_col[64:128, j : j + 1], op0=OP.mult, op1=OP.add,
                )
            for a in range(NK):
                for half in range(2):
                    nc.tensor.matmul(
                        psO[:, 512 * half : 512 * (half + 1)],
                        lhsT=ctx_bf[:, a : a + 1],
                        rhs=wo_all[:, D * a + 512 * half : D * a + 512 * (half + 1)],
                        start=(a == 0), stop=False,
                        skip_group_check=True,
                    )
            for half in range(2):
                nc.tensor.matmul(
                    psO[:, 512 * half : 512 * (half + 1)], lhsT=one_t[:, :],
                    rhs=bo_row[0:1, 512 * half : 512 * (half + 1)],
                    start=False, stop=True, skip_group_check=True,
                )
            out_sb = singles.tile([1, D], F32, tag="outsb")
            nc.scalar.copy(out=out_sb[:, :], in_=psO[:, :])
            nc.scalar.dma_start(out=out_e[:, :], in_=out_sb[:, :])

    nc.compile()
    return nc


def _pack128(a):
    # [1024, F] -> [128, 8*F] with out[p, k*F+i] = a[128k+p, i]
    rows, cols = a.shape
    return np.ascontiguousarray(
        a.reshape(NK, 128, cols).transpose(1, 0, 2).reshape(128, NK * cols)
    )


def _col(a):
    # [1024] -> [128, 8] with out[p, c] = a[128c+p]
    return np.ascontiguousarray(a.reshape(NK, 128).T)


def _prep_in_maps(inputs):
    bf = ml_dtypes.bfloat16
    f32 = np.float32

    def c(a, dt):
        return np.asarray(a, dtype=dt)

    x = c(inputs["x"], bf)
    smalls = np.concatenate(
        [
            _col(c(inputs["gamma"], f32)),
            _col(c(inputs["beta"], f32)),
            _col(c(inputs["b_q"], f32)),
            _col(c(inputs["b_v"], f32)),
        ],
        axis=1,
    )
    clsbo = np.stack([c(inputs["cls_token"], f32), c(inputs["b_o"], f32)])
    wkp = np.concatenate([_pack128(c(inputs["w_k"], bf)), _col(c(inputs["b_k"], bf))], axis=1)
    shared = {
        "gam": np.ascontiguousarray(c(inputs["gamma"], f32)),
        "bet": np.ascontiguousarray(c(inputs["beta"], f32)),
        "smalls": np.ascontiguousarray(smalls),
        "clsbo": np.ascontiguousarray(clsbo),
        "wqp": _pack128(c(np.asarray(inputs["w_q"]).T, bf)),
        "wkp": np.ascontiguousarray(wkp),
        "wvp": _pack128(c(np.asarray(inputs["w_v"]).T, bf)),
        "wop": _pack128(c(np.asarray(inputs["w_o"]).T, bf)),
    }
    return [{"x": _pack128(x[b]), **shared} for b in range(8)]


def run(inputs, trace=False, **kw):
    if "nc" not in _CACHE:
        _CACHE["nc"] = _build()
    nc = _CACHE["nc"]
    in_maps = _prep_in_maps(inputs)
    res = run_bass_kernel_spmd(nc, in_maps, core_ids=list(range(8)), trace=trace, **kw)
    out = np.stack([np.asarray(res.results[b]["out"][0], dtype=np.float32) for b in range(8)])
    return out, res


def kernel(**inputs):
    out, _ = run(inputs, trace=False)
    return out


# revision 12
# speedup vs baseline: 1.1748x; 1.1744x over previous
"""ABMIL (attention-based MIL) Trainium2 kernel, 8-core data-parallel.

Shapes (hardcoded): B=8 bags, N=1024 instances, D=1024, H=16 heads, hd=64.
Each NeuronCore processes one bag. Parameters are replicated; all big
tensors are host-packed into [128, F] layouts so every DMA is 128
contiguous descriptors (one per partition). All big HBM loads go on the
sync HWDGE ring in strict priority order (ring FIFO = priority): x first
(longest dependent chain), then w_q, w_k, w_v, w_o. No DMA transposes
(they entangle the scheduler's DMA-lane ordering with compute); the
znorm transpose runs on the PE with the gamma-fold applied during the
PSUM->SBUF copy.

Math (rank-1 attention: only the cls query row survives):
  znorm      = (z - mu) * rsqrt(var + eps)          rows of z = [cls; x_b]
  ln0        = znorm0 * gamma + beta
  q          = w_q @ ln0 + b_q
  U[h]       = w_k[64h:64h+64].T @ q[64h:64h+64]            (16 x 1024)
  scores     = 0.125*(U @ (gamma*znorm).T + U@beta + q_h@b_k_h)
  A          = softmax(scores)       (0.125 folded into the exp scale)
  Mrow       = (A @ znorm); ln-fold gamma/beta applied post-transpose
  ctx[64h:+64] = w_v[64h:+64] @ Mln[h] + b_v[64h:+64]
  out        = w_o @ ctx + b_o
"""

import numpy as np
import ml_dtypes

import concourse.bass as bass
import concourse.bacc as bacc
import concourse.mybir as mybir
import concourse.tile as tile
from concourse.bass_utils import run_bass_kernel_spmd

F32 = mybir.dt.float32
BF16 = mybir.dt.bfloat16
AX = mybir.AxisListType.X
OP = mybir.AluOpType
AF = mybir.ActivationFunctionType

D = 1024
NK = 8          # number of 128-chunks of D (and of x rows)
H = 16
EPS = 1e-5

_CACHE = {}


def _build():
    nc = bacc.Bacc()

    # Host-packed inputs: every [128, F] tensor DMAs as 128 contiguous rows.
    x_e = nc.declare_dram_parameter("x", [128, NK * D], BF16, isOutput=False)
    wq_e = nc.declare_dram_parameter("wqp", [128, NK * D], BF16, isOutput=False)
    # wk pack carries b_k (column layout) in its last 8 columns
    wk_e = nc.declare_dram_parameter("wkp", [128, NK * D + NK], BF16, isOutput=False)
    wv_e = nc.declare_dram_parameter("wvp", [128, NK * D], BF16, isOutput=False)
    wo_e = nc.declare_dram_parameter("wop", [128, NK * D], BF16, isOutput=False)
    # smalls: [gam_col, bet_col, bq_col, bv_col] each [128, 8] f32
    sm_e = nc.declare_dram_parameter("smalls", [128, 4 * NK], F32, isOutput=False)
    # clsbo: row 0 = cls_token, row 1 = b_o (both f32)
    cb_e = nc.declare_dram_parameter("clsbo", [2, D], F32, isOutput=False)
    gam_e = nc.declare_dram_parameter("gam", [D], F32, isOutput=False)
    bet_e = nc.declare_dram_parameter("bet", [D], F32, isOutput=False)
    out_e = nc.declare_dram_parameter("out", [1, D], F32, isOutput=True)

    with tile.TileContext(nc) as tc:
        with (
            tc.tile_pool(name="singles", bufs=1) as singles,
            tc.tile_pool(name="work", bufs=4) as work,
            tc.tile_pool(name="pt", bufs=2, space="PSUM") as pt,
            tc.tile_pool(name="pbig", bufs=2, space="PSUM") as pbig,
            tc.tile_pool(name="psm", bufs=2, space="PSUM") as psm,
        ):
            ident = singles.tile([128, 128], BF16, tag="ident")
            ident_dram = nc.inline_tensor(
                np.eye(128, dtype=ml_dtypes.bfloat16), name="ident_const"
            )

            eps_t = singles.tile([128, 1], F32, tag="eps")
            nc.vector.memset(eps_t[:, :], EPS)
            one_t = singles.tile([1, 1], F32, tag="one")
            nc.vector.memset(one_t[:, :], 1.0)

            cls_row = singles.tile([1, D], F32, tag="clsr")
            bo_row = singles.tile([1, D], F32, tag="bor")
            sm = singles.tile([128, 4 * NK], F32, tag="smalls")
            gam_col = sm[:, 0 * NK : 1 * NK]
            bet_col = sm[:, 1 * NK : 2 * NK]
            bq_col = sm[:, 2 * NK : 3 * NK]
            bv_col = sm[:, 3 * NK : 4 * NK]

            # big persistent tiles
            wq_all = singles.tile([128, NK * D], BF16, tag="wq")
            wk_all = singles.tile([128, NK * D + NK], BF16, tag="wk")
            bk_col = wk_all[:, NK * D : NK * D + NK]
            wv_all = singles.tile([128, NK * D], BF16, tag="wv")
            wo_all = singles.tile([128, NK * D], BF16, tag="wo")
            znorm_all = singles.tile([128, NK * D], BF16, tag="znorm")
            znT_all = singles.tile([128, NK * D], BF16, tag="znT")
            znT3 = znT_all[:, :].rearrange("p (c s) -> p c s", c=NK)
            xall = singles.tile([128, NK * D], BF16, tag="xall")
            xks = [xall[:, D * i : D * (i + 1)] for i in range(NK)]

            # --- DMA schedule -------------------------------------------
            # scalar ring: tiny loads + final out store.
            # sync ring: all big HBM loads, strict priority order.
            # (13 HWDGE DMAs; lane round-robin keeps compute-gated DMAs off
            #  the weight lanes.)
            nc.scalar.dma_start(out=cls_row[:, :], in_=cb_e[0:1, :])
            nc.scalar.dma_start(out=bo_row[:, :], in_=cb_e[1:2, :])
            nc.scalar.dma_start(out=sm[:, :], in_=sm_e[:, :])
            nc.scalar.dma_start(out=ident[:, :], in_=ident_dram[:, :])
            nc.sync.dma_start(out=xall[:, 0 : 4 * D], in_=x_e[:, 0 : 4 * D])
            nc.sync.dma_start(out=xall[:, 4 * D : 8 * D], in_=x_e[:, 4 * D : 8 * D])
            nc.sync.dma_start(out=wq_all[:, :], in_=wq_e[:, :])
            nc.sync.dma_start(out=wk_all[:, :], in_=wk_e[:, :])
            nc.sync.dma_start(out=wv_all[:, 0 : 4 * D], in_=wv_e[:, 0 : 4 * D])
            nc.sync.dma_start(out=wv_all[:, 4 * D : 8 * D], in_=wv_e[:, 4 * D : 8 * D])
            nc.sync.dma_start(out=wo_all[:, 0 : 4 * D], in_=wo_e[:, 0 : 4 * D])
            nc.sync.dma_start(out=wo_all[:, 4 * D : 8 * D], in_=wo_e[:, 4 * D : 8 * D])

            gam16 = singles.tile([H, D], F32, tag="gam16")
            nc.gpsimd.dma_start(
                out=gam16[:, :],
                in_=bass.AP(tensor=gam_e[:].tensor, offset=0, ap=[[0, H], [1, D]]),
            )
            bet16 = singles.tile([H, D], F32, tag="bet16")
            nc.gpsimd.dma_start(
                out=bet16[:, :],
                in_=bass.AP(tensor=bet_e[:].tensor, offset=0, ap=[[0, H], [1, D]]),
            )

            # ---- ACT table pre-warm (load all LUTs during DMA wait) -----
            warm = work.tile([1, 1], F32, tag="warm")
            nc.scalar.activation(out=warm[:, :], in_=eps_t[0:1, :], func=AF.Sqrt,
                                 bias=0.0, scale=1.0)
            nc.scalar.activation(out=warm[:, :], in_=eps_t[0:1, :], func=AF.Exp,
                                 bias=0.0, scale=1.0)
            nc.scalar.activation(out=warm[:, :], in_=eps_t[0:1, :], func=AF.Square,
                                 bias=0.0, scale=1.0)
            nc.scalar.activation(out=warm[:, :], in_=eps_t[0:1, :], func=AF.Copy,
                                 bias=0.0, scale=1.0)

            # ---- cls row LN --------------------------------------------
            stats0 = work.tile([1, 2, 6], F32, tag="stats0")
            nc.vector.bn_stats(out=stats0[:, 0, :], in_=cls_row[:, 0:512])
            nc.vector.bn_stats(out=stats0[:, 1, :], in_=cls_row[:, 512:1024])
            mv0 = work.tile([1, 2], F32, tag="mv0")
            nc.vector.bn_aggr(out=mv0[:, :], in_=stats0[:, :, :])
            nc.scalar.activation(
                out=mv0[:, 1:2], in_=mv0[:, 1:2], func=AF.Sqrt,
                bias=eps_t[0:1, :], scale=1.0,
            )
            rs0 = work.tile([1, 1], F32, tag="rs0")
            nc.vector.tensor_copy(out=rs0[:, :], in_=mv0[:, 1:2])
            nc.vector.reciprocal(out=rs0[:, :], in_=rs0[:, :])
            zn0_row = singles.tile([1, D], BF16, tag="zn0r")
            nc.vector.tensor_scalar(
                out=zn0_row[:, :], in0=cls_row[:, :],
                scalar1=mv0[:, 0:1], scalar2=rs0[:, 0:1],
                op0=OP.subtract, op1=OP.mult,
            )
            lzp = pt.tile([128, 16], BF16, tag="pt")
            for c in range(NK):
                nc.tensor.transpose(
                    out=lzp[:, 2 * c : 2 * c + 1],
                    in_=zn0_row[0:1, 128 * c : 128 * (c + 1)],
                    identity=ident[0:1, 0:1],
                )
            zn0_col = singles.tile([128, NK], BF16, tag="zn0c")
            nc.scalar.copy(
                out=zn0_col[:, :],
                in_=lzp[:, :].rearrange("p (c x) -> p c x", c=NK)[:, :, 0],
            )
            ln0_col = singles.tile([128, NK], BF16, tag="ln0c")
            nc.vector.tensor_mul(out=ln0_col[:, :], in0=zn0_col[:, :], in1=gam_col[:, :])
            nc.vector.tensor_add(out=ln0_col[:, :], in0=ln0_col[:, :], in1=bet_col[:, :])

            # ---- x LayerNorm pipeline: stats -> normalize -> transpose --
            # (emitted before weight-dependent ops so DVE/ACT streams never
            #  block on weights; transposes on the PE, gamma folded into the
            #  PSUM->SBUF copy on ACT)
            for k in range(NK):
                xk = xks[k]
                stats = work.tile([128, 2, 6], F32, tag="stats", name=f"stats{k}")
                nc.vector.bn_stats(out=stats[:, 0, :], in_=xk[:, 0:512])
                nc.vector.bn_stats(out=stats[:, 1, :], in_=xk[:, 512:1024])
                mv = work.tile([128, 2], F32, tag="mv", name=f"mv{k}")
                nc.vector.bn_aggr(out=mv[:, :], in_=stats[:, :, :])
                nc.scalar.activation(
                    out=mv[:, 1:2], in_=mv[:, 1:2], func=AF.Sqrt,
                    bias=eps_t[:, :], scale=1.0,
                )
                rs = work.tile([128, 1], F32, tag="rs", name=f"rs{k}")
                nc.vector.tensor_copy(out=rs[:, :], in_=mv[:, 1:2])
                nc.vector.reciprocal(out=rs[:, :], in_=rs[:, :])
                nc.vector.tensor_scalar(
                    out=znorm_all[:, D * k : D * (k + 1)], in0=xk[:, :],
                    scalar1=mv[:, 0:1], scalar2=rs[:, 0:1],
                    op0=OP.subtract, op1=OP.mult,
                )
                ptb = pt.tile([128, D], BF16, tag="pt", name=f"ptb{k}")
                for j in range(NK):
                    nc.tensor.transpose(
                        out=ptb[:, 128 * j : 128 * (j + 1)],
                        in_=znorm_all[:, D * k + 128 * j : D * k + 128 * (j + 1)],
                        identity=ident[:, :],
                    )
                if k % 2 == 0:
                    nc.scalar.copy(
                        out=znT3[:, :, 128 * k : 128 * (k + 1)],
                        in_=ptb[:, :].rearrange("p (c s) -> p c s", c=NK),
                    )
                else:
                    nc.vector.tensor_copy(
                        out=znT3[:, :, 128 * k : 128 * (k + 1)],
                        in_=ptb[:, :].rearrange("p (c s) -> p c s", c=NK),
                    )

            # ---- q = ln0 @ w_q.T + b_q ---------------------------------
            psq = pbig.tile([1, D], F32, tag="pbig")
            for c in range(NK):
                for half in range(2):
                    nc.tensor.matmul(
                        psq[:, 512 * half : 512 * (half + 1)], lhsT=ln0_col[:, c : c + 1],
                        rhs=wq_all[:, D * c + 512 * half : D * c + 512 * (half + 1)],
                        start=(c == 0), stop=(c == NK - 1),
                        skip_group_check=True,
                    )
            q_sb = singles.tile([1, D], BF16, tag="qsb")
            nc.scalar.copy(out=q_sb[:, :], in_=psq[:, :])
            qcp = pt.tile([128, 16], BF16, tag="pt")
            for c in range(NK):
                nc.tensor.transpose(
                    out=qcp[:, 2 * c : 2 * c + 1],
                    in_=q_sb[0:1, 128 * c : 128 * (c + 1)],
                    identity=ident[0:1, 0:1],
                )
            q_col = singles.tile([128, NK], BF16, tag="qcol")
            nc.scalar.copy(
                out=q_col[:, :],
                in_=qcp[:, :].rearrange("p (c x) -> p c x", c=NK)[:, :, 0],
            )
            nc.vector.tensor_add(out=q_col[:, :], in0=q_col[:, :], in1=bq_col[:, :])
            qbT = singles.tile([128, H * NK], BF16, tag="qbT")
            nc.gpsimd.memset(qbT[:, :], 0.0)
            for c in range(NK):
                nc.gpsimd.tensor_copy(
                    out=qbT[0:64, H * c + 2 * c : H * c + 2 * c + 1],
                    in_=q_col[0:64, c : c + 1],
                )
                nc.gpsimd.tensor_copy(
                    out=qbT[64:128, H * c + 2 * c + 1 : H * c + 2 * c + 2],
                    in_=q_col[64:128, c : c + 1],
                )

            # ---- U = Qblk @ w_k ; e, softmax shift ---------------------
            psU = pbig.tile([H, D], F32, tag="pbig")
            for c in range(NK):
                for half in range(2):
                    nc.tensor.matmul(
                        psU[:, 512 * half : 512 * (half + 1)], lhsT=qbT[:, H * c : H * (c + 1)],
                        rhs=wk_all[:, D * c + 512 * half : D * c + 512 * (half + 1)],
                        start=(c == 0), stop=(c == NK - 1),
                        skip_group_check=True,
                    )
            pse2 = psm.tile([H, 1], F32, tag="psm")
            for c in range(NK):
                nc.tensor.matmul(
                    pse2[:, :], lhsT=qbT[:, H * c : H * (c + 1)], rhs=bk_col[:, c : c + 1],
                    start=(c == 0), stop=(c == NK - 1),
                )
            # Ug = U * gamma in bf16 (0.125 folded into the exp scale)
            ug = singles.tile([H, D], BF16, tag="ug")
            nc.vector.tensor_mul(out=ug[:, :], in0=psU[:, :], in1=gam16[:, :])
            # e1 = sum_d U*beta  (fused multiply+reduce on DVE)
            ttr_o = work.tile([H, D], F32, tag="ttro")
            nc.vector.tensor_mul(out=ttr_o[:, :], in0=psU[:, :], in1=bet16[:, :])
            e1 = work.tile([H, 1], F32, tag="e1")
            nc.vector.reduce_sum(out=e1[:, :], in_=ttr_o[:, :], axis=AX)
            e_sb = singles.tile([H, 1], F32, tag="esb")
            nc.vector.tensor_add(out=e_sb[:, :], in0=e1[:, :], in1=pse2[:, :])
            nc.vector.tensor_scalar_mul(out=e_sb[:, :], in0=e_sb[:, :], scalar1=0.125)

            ugp = pt.tile([128, 128], BF16, tag="pt")
            for c in range(NK):
                nc.tensor.transpose(
                    out=ugp[:, H * c : H * (c + 1)], in_=ug[:, 128 * c : 128 * (c + 1)],
                    identity=ident[0:H, 0:H],
                )
            ugT = singles.tile([128, H * NK], BF16, tag="ugT")
            nc.scalar.copy(out=ugT[:, :], in_=ugp[:, :])

            # safe softmax shift: bound_h = 0.125*||U_h||*max|g|*32 >= max score
            # s2 = sum U^2 via ACT Square+accum; bound = sqrt(s2 * (4 max|g|)^2)
            sq16 = work.tile([H, D], F32, tag="sq16")
            nc.vector.tensor_mul(out=sq16[:, :], in0=ug[:, :], in1=ug[:, :])
            s2 = work.tile([H, 1], F32, tag="s2")
            nc.vector.reduce_sum(out=s2[:, :], in_=sq16[:, :], axis=AX)
            bound = work.tile([H, 1], F32, tag="bound")
            nc.scalar.activation(
                out=bound[:, :], in_=s2[:, :], func=AF.Sqrt, bias=0.0, scale=16.0
            )
            eb = work.tile([H, 1], F32, tag="eb")
            nc.vector.tensor_sub(out=eb[:, :], in0=e_sb[:, :], in1=bound[:, :])

            # ---- cls score / attention prologue ------------------------
            a_sb = singles.tile([H, 1025], BF16, tag="asb")
            aT = singles.tile([128, H * NK], BF16, tag="aT")
            se = work.tile([H, 3], F32, tag="seall")
            se0 = se[:, 2:3]
            psS0 = psm.tile([H, 1], F32, tag="psm")
            for c in range(NK):
                nc.tensor.matmul(
                    psS0[:, :], lhsT=ugT[:, H * c : H * (c + 1)], rhs=zn0_col[:, c : c + 1],
                    start=(c == 0), stop=(c == NK - 1),
                )
            nc.scalar.activation(
                out=a_sb[:, 0:1], in_=psS0[:, :], func=AF.Exp,
                bias=eb[:, 0:1], scale=0.125, accum_out=se0,
            )
            a0p = pt.tile([128, 16], BF16, tag="pt")
            nc.tensor.transpose(out=a0p[0:1, 0:H], in_=a_sb[:, 0:1], identity=ident[0:H, 0:H])
            aT0 = singles.tile([1, H], BF16, tag="aT0")
            nc.scalar.copy(out=aT0[:, :], in_=a0p[0:1, 0:H])
            psM = pbig.tile([H, D], F32, tag="pbig")
            for half in range(2):
                nc.tensor.matmul(
                    psM[:, 512 * half : 512 * (half + 1)], lhsT=aT0[:, :],
                    rhs=zn0_row[:, 512 * half : 512 * (half + 1)],
                    start=True, stop=False, skip_group_check=True,
                )

            # ---- scores/softmax per 512-col half, then Mrow ------------
            for half in range(2):
                psS = psm.tile([H, 512], F32, tag="psm", name=f"psS{half}")
                for c in range(NK):
                    nc.tensor.matmul(
                        psS[:, :], lhsT=ugT[:, H * c : H * (c + 1)],
                        rhs=znT_all[:, D * c + 512 * half : D * c + 512 * (half + 1)],
                        start=(c == 0), stop=(c == NK - 1),
                    )
                nc.scalar.activation(
                    out=a_sb[:, 1 + 512 * half : 1 + 512 * (half + 1)], in_=psS[:, :],
                    func=AF.Exp, bias=eb[:, 0:1], scale=0.125,
                    accum_out=se[:, half : half + 1],
                )
                for kk in range(4):
                    k = 4 * half + kk
                    atpk = pt.tile([128, 16], BF16, tag="pt", name=f"atp{k}")
                    nc.tensor.transpose(
                        out=atpk[:, 0:H],
                        in_=a_sb[:, 1 + 128 * k : 1 + 128 * (k + 1)],
                        identity=ident[0:H, 0:H],
                    )
                    nc.scalar.copy(out=aT[:, H * k : H * (k + 1)], in_=atpk[:, 0:H])
                    for dh in range(2):
                        nc.tensor.matmul(
                            psM[:, 512 * dh : 512 * (dh + 1)], lhsT=aT[:, H * k : H * (k + 1)],
                            rhs=znorm_all[:, D * k + 512 * dh : D * k + 512 * (dh + 1)],
                            start=False, stop=(k == NK - 1), skip_group_check=True,
                        )

            rinv = work.tile([H, 1], F32, tag="rinv")
            nc.vector.reduce_sum(out=rinv[:, :], in_=se[:, :], axis=AX)
            nc.vector.reciprocal(out=rinv[:, :], in_=rinv[:, :])

            # mrow = psM * rinv  (gamma/beta fold applied after transpose)
            mrow = singles.tile([H, D], BF16, tag="mrow")
            nc.vector.tensor_scalar_mul(out=mrow[:, :], in0=psM[:, :], scalar1=rinv[:, 0:1])
            mT = singles.tile([128, H * NK], BF16, tag="mT")
            for c in range(NK):
                mtp = pt.tile([128, 16], BF16, tag="pt", name=f"mtp{c}")
                nc.tensor.transpose(
                    out=mtp[:, 0:H], in_=mrow[:, 128 * c : 128 * (c + 1)],
                    identity=ident[0:H, 0:H],
                )
                # fold PSUM->SBUF copy into the gamma*x+beta apply
                nc.vector.tensor_scalar(
                    out=mT[:, H * c : H * (c + 1)], in0=mtp[:, 0:H],
                    scalar1=gam_col[:, c : c + 1], scalar2=bet_col[:, c : c + 1],
                    op0=OP.mult, op1=OP.add,
                )

            # ---- ctx via [16,512] trick + transpose-select --------------
            # ps16[h, n] = sum_d Mln[h, d] * w_v[n, d]; ctx[n] = ps16[n//64, n]
            # c-outer so chunks 0-3 start when the first wv half lands
            ps16 = pbig.tile([H, D], F32, tag="pbig")
            for c in range(NK):
                for half in range(2):
                    nc.tensor.matmul(
                        ps16[:, 512 * half : 512 * (half + 1)], lhsT=mT[:, H * c : H * (c + 1)],
                        rhs=wv_all[:, D * c + 512 * half : D * c + 512 * (half + 1)],
                        start=(c == 0), stop=(c == NK - 1),
                        skip_group_check=True,
                    )
            c16 = singles.tile([H, D], BF16, tag="c16")
            nc.scalar.copy(out=c16[:, :], in_=ps16[:, :])
            ctx_bf = singles.tile([128, NK], BF16, tag="ctxbf")
            psO = pbig.tile([1, D], F32, tag="pbig")
            for j in range(NK):
                ctp = pt.tile([128, 16], BF16, tag="pt", name=f"ctp{j}")
                nc.tensor.transpose(
                    out=ctp[:, 0:H], in_=c16[:, 128 * j : 128 * (j + 1)],
                    identity=ident[0:H, 0:H],
                )
                nc.vector.scalar_tensor_tensor(
                    out=ctx_bf[0:64, j : j + 1], in0=ctp[0:64, 2 * j : 2 * j + 1],
                    scalar=1.0, in1=bv_col[0:64, j : j + 1], op0=OP.mult, op1=OP.add,
                )
                nc.vector.scalar_tensor_tensor(
                    out=ctx_bf[64:128, j : j + 1], in0=ctp[64:128, 2 * j + 1 : 2 * j + 2],
                    scalar=1.0, in1=bv_col[64:128, j : j + 1], op0=OP.mult, op1=OP.add,
                )
            for a in range(NK):
                for half in range(2):
                    nc.tensor.matmul(
                        psO[:, 512 * half : 512 * (half + 1)],
                        lhsT=ctx_bf[:, a : a + 1],
                        rhs=wo_all[:, D * a + 512 * half : D * a + 512 * (half + 1)],
                        start=(a == 0), stop=False,
                        skip_group_check=True,
                    )
            for half in range(2):
                nc.tensor.matmul(
                    psO[:, 512 * half : 512 * (half + 1)], lhsT=one_t[:, :],
                    rhs=bo_row[0:1, 512 * half : 512 * (half + 1)],
                    start=False, stop=True, skip_group_check=True,
                )
            out_sb = singles.tile([1, D], F32, tag="outsb")
            nc.scalar.copy(out=out_sb[:, :], in_=psO[:, :])
            nc.scalar.dma_start(out=out_e[:, :], in_=out_sb[:, :])

    nc.compile()
    return nc


def _pack128(a):
    rows, cols = a.shape
    return np.ascontiguousarray(
        a.reshape(NK, 128, cols).transpose(1, 0, 2).reshape(128, NK * cols)
    )


def _col(a):
    return np.ascontiguousarray(a.reshape(NK, 128).T)


def _prep_in_maps(inputs):
    bf = ml_dtypes.bfloat16
    f32 = np.float32

    def c(a, dt):
        return np.asarray(a, dtype=dt)

    x = c(inputs["x"], bf)
    smalls = np.concatenate(
        [
            _col(c(inputs["gamma"], f32)),
            _col(c(inputs["beta"], f32)),
            _col(c(inputs["b_q"], f32)),
            _col(c(inputs["b_v"], f32)),
        ],
        axis=1,
    )
    clsbo = np.stack([c(inputs["cls_token"], f32), c(inputs["b_o"], f32)])
    wkp = np.concatenate([_pack128(c(inputs["w_k"], bf)), _col(c(inputs["b_k"], bf))], axis=1)
    shared = {
        "gam": np.ascontiguousarray(c(inputs["gamma"], f32)),
        "bet": np.ascontiguousarray(c(inputs["beta"], f32)),
        "smalls": np.ascontiguousarray(smalls),
        "clsbo": np.ascontiguousarray(clsbo),
        "wqp": _pack128(c(np.asarray(inputs["w_q"]).T, bf)),
        "wkp": np.ascontiguousarray(wkp),
        "wvp": _pack128(c(np.asarray(inputs["w_v"]).T, bf)),
        "wop": _pack128(c(np.asarray(inputs["w_o"]).T, bf)),
    }
    return [{"x": _pack128(x[b]), **shared} for b in range(8)]


def run(inputs, trace=False, **kw):
    if "nc" not in _CACHE:
        _CACHE["nc"] = _build()
    nc = _CACHE["nc"]
    in_maps = _prep_in_maps(inputs)
    res = run_bass_kernel_spmd(nc, in_maps, core_ids=list(range(8)), trace=trace, **kw)
    out = np.stack([np.asarray(res.results[b]["out"][0], dtype=np.float32) for b in range(8)])
    return out, res


def kernel(**inputs):
    out, _ = run(inputs, trace=False)
    return out


# revision 13
# speedup vs baseline: 1.3024x; 1.1085x over previous
"""ABMIL (attention-based MIL) Trainium2 kernel, 8-core data-parallel.

Shapes (hardcoded): B=8 bags, N=1024 instances, D=1024, H=16 heads, hd=64.
Each NeuronCore processes one bag. Parameters are replicated; all big
tensors are host-packed into [128, F] layouts so every DMA is 128
contiguous descriptors (one per partition). All big HBM loads go on the
sync HWDGE ring in strict priority order (ring FIFO = priority): x first
(longest dependent chain), then w_q, w_k, w_v, w_o. No DMA transposes
(they entangle the scheduler's DMA-lane ordering with compute); the
znorm transpose runs on the PE with the gamma-fold applied during the
PSUM->SBUF copy.

Math (rank-1 attention: only the cls query row survives):
  znorm      = (z - mu) * rsqrt(var + eps)          rows of z = [cls; x_b]
  ln0        = znorm0 * gamma + beta
  q          = w_q @ ln0 + b_q
  U[h]       = w_k[64h:64h+64].T @ q[64h:64h+64]            (16 x 1024)
  scores     = 0.125*(U @ (gamma*znorm).T + U@beta + q_h@b_k_h)
  A          = softmax(scores)       (0.125 folded into the exp scale)
  Mrow       = (A @ znorm); ln-fold gamma/beta applied post-transpose
  ctx[64h:+64] = w_v[64h:+64] @ Mln[h] + b_v[64h:+64]
  out        = w_o @ ctx + b_o
"""

import numpy as np
import ml_dtypes

import concourse.bass as bass
import concourse.bacc as bacc
import concourse.mybir as mybir
import concourse.tile as tile
from concourse.bass_utils import run_bass_kernel_spmd

F32 = mybir.dt.float32
BF16 = mybir.dt.bfloat16
AX = mybir.AxisListType.X
OP = mybir.AluOpType
AF = mybir.ActivationFunctionType

D = 1024
NK = 8          # number of 128-chunks of D (and of x rows)
H = 16
EPS = 1e-5

_CACHE = {}


def _build():
    nc = bacc.Bacc()

    # Host-packed inputs: every [128, F] tensor DMAs as 128 contiguous rows.
    x_e = nc.declare_dram_parameter("x", [128, NK * D], BF16, isOutput=False)
    wq_e = nc.declare_dram_parameter("wqp", [128, NK * D], BF16, isOutput=False)
    # wk pack carries b_k (column layout) in its last 8 columns
    wk_e = nc.declare_dram_parameter("wkp", [128, NK * D + NK], BF16, isOutput=False)
    wv_e = nc.declare_dram_parameter("wvp", [128, NK * D], BF16, isOutput=False)
    wo_e = nc.declare_dram_parameter("wop", [128, NK * D], BF16, isOutput=False)
    # smalls: [gam_col, bet_col, bq_col, bv_col] each [128, 8] f32
    sm_e = nc.declare_dram_parameter("smalls", [128, 4 * NK], F32, isOutput=False)
    # clsbo: row 0 = cls_token, row 1 = b_o (both f32)
    cb_e = nc.declare_dram_parameter("clsbo", [2, D], F32, isOutput=False)
    gam_e = nc.declare_dram_parameter("gam", [D], F32, isOutput=False)
    bet_e = nc.declare_dram_parameter("bet", [D], F32, isOutput=False)
    out_e = nc.declare_dram_parameter("out", [1, D], F32, isOutput=True)

    with tile.TileContext(nc) as tc:
        with (
            tc.tile_pool(name="singles", bufs=1) as singles,
            tc.tile_pool(name="work", bufs=4) as work,
            tc.tile_pool(name="pt", bufs=2, space="PSUM") as pt,
            tc.tile_pool(name="pbig", bufs=2, space="PSUM") as pbig,
            tc.tile_pool(name="psm", bufs=2, space="PSUM") as psm,
        ):
            ident = singles.tile([128, 128], BF16, tag="ident")
            ident_dram = nc.inline_tensor(
                np.eye(128, dtype=ml_dtypes.bfloat16), name="ident_const"
            )

            eps_t = singles.tile([128, 1], F32, tag="eps")
            nc.vector.memset(eps_t[:, :], EPS)
            one_t = singles.tile([1, 1], F32, tag="one")
            nc.vector.memset(one_t[:, :], 1.0)

            cls_row = singles.tile([1, D], F32, tag="clsr")
            bo_row = singles.tile([1, D], F32, tag="bor")
            sm = singles.tile([128, 4 * NK], F32, tag="smalls")
            gam_col = sm[:, 0 * NK : 1 * NK]
            bet_col = sm[:, 1 * NK : 2 * NK]
            bq_col = sm[:, 2 * NK : 3 * NK]
            bv_col = sm[:, 3 * NK : 4 * NK]

            # big persistent tiles
            wq_all = singles.tile([128, NK * D], BF16, tag="wq")
            wk_all = singles.tile([128, NK * D + NK], BF16, tag="wk")
            bk_col = wk_all[:, NK * D : NK * D + NK]
            wv_all = singles.tile([128, NK * D], BF16, tag="wv")
            wo_all = singles.tile([128, NK * D], BF16, tag="wo")
            znorm_all = singles.tile([128, NK * D], BF16, tag="znorm")
            znT_all = singles.tile([128, NK * D], BF16, tag="znT")
            znT3 = znT_all[:, :].rearrange("p (c s) -> p c s", c=NK)
            xq0 = singles.tile([128, 2 * D], BF16, tag="xq0")
            xq1 = singles.tile([128, 2 * D], BF16, tag="xq1")
            xq2 = singles.tile([128, 2 * D], BF16, tag="xq2")
            xq3 = singles.tile([128, 2 * D], BF16, tag="xq3")
            xqs = [xq0, xq1, xq2, xq3]
            xks = [xqs[i // 2][:, D * (i % 2) : D * (i % 2 + 1)] for i in range(NK)]

            # --- DMA schedule -------------------------------------------
            # scalar ring: tiny loads + final out store.
            # sync ring: all big HBM loads, strict priority order.
            # (pure-data DMAs only on HWDGE lanes - no compute-gated DMAs)
            warm = work.tile([1, 1], F32, tag="warm")
            nc.scalar.activation(out=warm[:, :], in_=eps_t[0:1, :], func=AF.Sqrt,
                                 bias=0.0, scale=1.0)
            nc.scalar.dma_start(out=cls_row[:, :], in_=cb_e[0:1, :])
            nc.scalar.dma_start(out=bo_row[:, :], in_=cb_e[1:2, :])
            nc.scalar.dma_start(out=sm[:, :], in_=sm_e[:, :])
            nc.scalar.dma_start(out=ident[:, :], in_=ident_dram[:, :])
            for i in range(4):
                nc.sync.dma_start(
                    out=xqs[i][:, :], in_=x_e[:, 2 * D * i : 2 * D * (i + 1)]
                )
            nc.sync.dma_start(out=wq_all[:, :], in_=wq_e[:, :])
            nc.sync.dma_start(out=wk_all[:, :], in_=wk_e[:, :])
            nc.sync.dma_start(out=wv_all[:, 0 : 4 * D], in_=wv_e[:, 0 : 4 * D])
            nc.sync.dma_start(out=wv_all[:, 4 * D : 8 * D], in_=wv_e[:, 4 * D : 8 * D])
            nc.sync.dma_start(out=wo_all[:, 0 : 4 * D], in_=wo_e[:, 0 : 4 * D])
            nc.sync.dma_start(out=wo_all[:, 4 * D : 8 * D], in_=wo_e[:, 4 * D : 8 * D])

            gam16 = singles.tile([H, D], F32, tag="gam16")
            nc.gpsimd.dma_start(
                out=gam16[:, :],
                in_=bass.AP(tensor=gam_e[:].tensor, offset=0, ap=[[0, H], [1, D]]),
            )
            bet16 = singles.tile([H, D], F32, tag="bet16")
            nc.gpsimd.dma_start(
                out=bet16[:, :],
                in_=bass.AP(tensor=bet_e[:].tensor, offset=0, ap=[[0, H], [1, D]]),
            )

            # remaining ACT table pre-warms (Exp/Copy LUTs load during DMA)
            nc.scalar.activation(out=warm[:, :], in_=eps_t[0:1, :], func=AF.Exp,
                                 bias=0.0, scale=1.0)
            nc.scalar.activation(out=warm[:, :], in_=eps_t[0:1, :], func=AF.Copy,
                                 bias=0.0, scale=1.0)

            # ---- cls row LN --------------------------------------------
            stats0 = work.tile([1, 2, 6], F32, tag="stats0")
            nc.vector.bn_stats(out=stats0[:, 0, :], in_=cls_row[:, 0:512])
            nc.vector.bn_stats(out=stats0[:, 1, :], in_=cls_row[:, 512:1024])
            mv0 = work.tile([1, 2], F32, tag="mv0")
            nc.vector.bn_aggr(out=mv0[:, :], in_=stats0[:, :, :])
            nc.scalar.activation(
                out=mv0[:, 1:2], in_=mv0[:, 1:2], func=AF.Sqrt,
                bias=eps_t[0:1, :], scale=1.0,
            )
            rs0 = work.tile([1, 1], F32, tag="rs0")
            nc.vector.reciprocal(out=rs0[:, :], in_=mv0[:, 1:2])
            zn0_row = singles.tile([1, D], BF16, tag="zn0r")
            nc.vector.tensor_scalar(
                out=zn0_row[:, :], in0=cls_row[:, :],
                scalar1=mv0[:, 0:1], scalar2=rs0[:, 0:1],
                op0=OP.subtract, op1=OP.mult,
            )
            lzp = pt.tile([128, 16], BF16, tag="pt")
            for c in range(NK):
                nc.tensor.transpose(
                    out=lzp[:, 2 * c : 2 * c + 1],
                    in_=zn0_row[0:1, 128 * c : 128 * (c + 1)],
                    identity=ident[0:1, 0:1],
                )
            zn0_col = singles.tile([128, NK], BF16, tag="zn0c")
            nc.scalar.copy(
                out=zn0_col[:, :],
                in_=lzp[:, :].rearrange("p (c x) -> p c x", c=NK)[:, :, 0],
            )
            ln0_col = singles.tile([128, NK], BF16, tag="ln0c")
            nc.vector.tensor_mul(out=ln0_col[:, :], in0=zn0_col[:, :], in1=gam_col[:, :])
            nc.vector.tensor_add(out=ln0_col[:, :], in0=ln0_col[:, :], in1=bet_col[:, :])

            # ---- x LayerNorm pipeline: stats -> normalize -> transpose --
            # (emitted before weight-dependent ops so DVE/ACT streams never
            #  block on weights; transposes on the PE, gamma folded into the
            #  PSUM->SBUF copy on ACT)
            for k in range(NK):
                xk = xks[k]
                stats = work.tile([128, 2, 6], F32, tag="stats", name=f"stats{k}")
                nc.vector.bn_stats(out=stats[:, 0, :], in_=xk[:, 0:512])
                nc.vector.bn_stats(out=stats[:, 1, :], in_=xk[:, 512:1024])
                mv = work.tile([128, 2], F32, tag="mv", name=f"mv{k}")
                nc.vector.bn_aggr(out=mv[:, :], in_=stats[:, :, :])
                nc.scalar.activation(
                    out=mv[:, 1:2], in_=mv[:, 1:2], func=AF.Sqrt,
                    bias=eps_t[:, :], scale=1.0,
                )
                rs = work.tile([128, 1], F32, tag="rs", name=f"rs{k}")
                nc.vector.reciprocal(out=rs[:, :], in_=mv[:, 1:2])
                nc.vector.tensor_scalar(
                    out=znorm_all[:, D * k : D * (k + 1)], in0=xk[:, :],
                    scalar1=mv[:, 0:1], scalar2=rs[:, 0:1],
                    op0=OP.subtract, op1=OP.mult,
                )
                ptb = pt.tile([128, D], BF16, tag="pt", name=f"ptb{k}")
                for j in range(NK):
                    nc.tensor.transpose(
                        out=ptb[:, 128 * j : 128 * (j + 1)],
                        in_=znorm_all[:, D * k + 128 * j : D * k + 128 * (j + 1)],
                        identity=ident[:, :],
                    )
                if k % 2 == 0:
                    nc.scalar.copy(
                        out=znT3[:, :, 128 * k : 128 * (k + 1)],
                        in_=ptb[:, :].rearrange("p (c s) -> p c s", c=NK),
                    )
                else:
                    nc.vector.tensor_copy(
                        out=znT3[:, :, 128 * k : 128 * (k + 1)],
                        in_=ptb[:, :].rearrange("p (c s) -> p c s", c=NK),
                    )

            # ---- q = ln0 @ w_q.T + b_q ---------------------------------
            psq = pbig.tile([1, D], F32, tag="pbig")
            for c in range(NK):
                for half in range(2):
                    nc.tensor.matmul(
                        psq[:, 512 * half : 512 * (half + 1)], lhsT=ln0_col[:, c : c + 1],
                        rhs=wq_all[:, D * c + 512 * half : D * c + 512 * (half + 1)],
                        start=(c == 0), stop=(c == NK - 1),
                        skip_group_check=True,
                    )
            q_sb = singles.tile([1, D], BF16, tag="qsb")
            nc.scalar.copy(out=q_sb[:, :], in_=psq[:, :])
            qcp = pt.tile([128, 16], BF16, tag="pt")
            for c in range(NK):
                nc.tensor.transpose(
                    out=qcp[:, 2 * c : 2 * c + 1],
                    in_=q_sb[0:1, 128 * c : 128 * (c + 1)],
                    identity=ident[0:1, 0:1],
                )
            q_col = singles.tile([128, NK], BF16, tag="qcol")
            nc.scalar.copy(
                out=q_col[:, :],
                in_=qcp[:, :].rearrange("p (c x) -> p c x", c=NK)[:, :, 0],
            )
            nc.vector.tensor_add(out=q_col[:, :], in0=q_col[:, :], in1=bq_col[:, :])
            qbT = singles.tile([128, H * NK], BF16, tag="qbT")
            nc.gpsimd.memset(qbT[:, :], 0.0)
            for c in range(NK):
                nc.gpsimd.tensor_copy(
                    out=qbT[0:64, H * c + 2 * c : H * c + 2 * c + 1],
                    in_=q_col[0:64, c : c + 1],
                )
                nc.gpsimd.tensor_copy(
                    out=qbT[64:128, H * c + 2 * c + 1 : H * c + 2 * c + 2],
                    in_=q_col[64:128, c : c + 1],
                )

            # ---- U = Qblk @ w_k ; e, softmax shift ---------------------
            psU = pbig.tile([H, D], F32, tag="pbig")
            for c in range(NK):
                for half in range(2):
                    nc.tensor.matmul(
                        psU[:, 512 * half : 512 * (half + 1)], lhsT=qbT[:, H * c : H * (c + 1)],
                        rhs=wk_all[:, D * c + 512 * half : D * c + 512 * (half + 1)],
                        start=(c == 0), stop=(c == NK - 1),
                        skip_group_check=True,
                    )
            pse2 = psm.tile([H, 1], F32, tag="psm")
            for c in range(NK):
                nc.tensor.matmul(
                    pse2[:, :], lhsT=qbT[:, H * c : H * (c + 1)], rhs=bk_col[:, c : c + 1],
                    start=(c == 0), stop=(c == NK - 1),
                )
            # Ug = U * gamma in bf16 (0.125 folded into the exp scale)
            ug = singles.tile([H, D], BF16, tag="ug")
            nc.vector.tensor_mul(out=ug[:, :], in0=psU[:, :], in1=gam16[:, :])
            # e1 = sum_d U*beta  (fused multiply+reduce on DVE)
            ttr_o = work.tile([H, D], F32, tag="ttro")
            nc.vector.tensor_mul(out=ttr_o[:, :], in0=psU[:, :], in1=bet16[:, :])
            e1 = work.tile([H, 1], F32, tag="e1")
            nc.vector.reduce_sum(out=e1[:, :], in_=ttr_o[:, :], axis=AX)
            e_sb = singles.tile([H, 1], F32, tag="esb")
            nc.vector.tensor_add(out=e_sb[:, :], in0=e1[:, :], in1=pse2[:, :])
            nc.vector.tensor_scalar_mul(out=e_sb[:, :], in0=e_sb[:, :], scalar1=0.125)

            ugp = pt.tile([128, 128], BF16, tag="pt")
            for c in range(NK):
                nc.tensor.transpose(
                    out=ugp[:, H * c : H * (c + 1)], in_=ug[:, 128 * c : 128 * (c + 1)],
                    identity=ident[0:H, 0:H],
                )
            ugT = singles.tile([128, H * NK], BF16, tag="ugT")
            nc.scalar.copy(out=ugT[:, :], in_=ugp[:, :])

            # safe softmax shift: bound_h = 0.125*||U_h||*max|g|*32 >= max score
            # s2 = sum U^2 via ACT Square+accum; bound = sqrt(s2 * (4 max|g|)^2)
            sq16 = work.tile([H, D], F32, tag="sq16")
            nc.vector.tensor_mul(out=sq16[:, :], in0=ug[:, :], in1=ug[:, :])
            s2 = work.tile([H, 1], F32, tag="s2")
            nc.vector.reduce_sum(out=s2[:, :], in_=sq16[:, :], axis=AX)
            bound = work.tile([H, 1], F32, tag="bound")
            nc.scalar.activation(
                out=bound[:, :], in_=s2[:, :], func=AF.Sqrt, bias=0.0, scale=16.0
            )
            eb = work.tile([H, 1], F32, tag="eb")
            nc.vector.tensor_sub(out=eb[:, :], in0=e_sb[:, :], in1=bound[:, :])

            # ---- cls score / attention prologue ------------------------
            a_sb = singles.tile([H, 1025], BF16, tag="asb")
            aT = singles.tile([128, H * NK], BF16, tag="aT")
            se = work.tile([H, 3], F32, tag="seall")
            se0 = se[:, 2:3]
            psS0 = psm.tile([H, 1], F32, tag="psm")
            for c in range(NK):
                nc.tensor.matmul(
                    psS0[:, :], lhsT=ugT[:, H * c : H * (c + 1)], rhs=zn0_col[:, c : c + 1],
                    start=(c == 0), stop=(c == NK - 1),
                )
            nc.scalar.activation(
                out=a_sb[:, 0:1], in_=psS0[:, :], func=AF.Exp,
                bias=eb[:, 0:1], scale=0.125, accum_out=se0,
            )
            a0p = pt.tile([128, 16], BF16, tag="pt")
            nc.tensor.transpose(out=a0p[0:1, 0:H], in_=a_sb[:, 0:1], identity=ident[0:H, 0:H])
            aT0 = singles.tile([1, H], BF16, tag="aT0")
            nc.scalar.copy(out=aT0[:, :], in_=a0p[0:1, 0:H])
            psM = pbig.tile([H, D], F32, tag="pbig")
            for half in range(2):
                nc.tensor.matmul(
                    psM[:, 512 * half : 512 * (half + 1)], lhsT=aT0[:, :],
                    rhs=zn0_row[:, 512 * half : 512 * (half + 1)],
                    start=True, stop=False, skip_group_check=True,
                )

            # ---- scores/softmax per 512-col half, then Mrow ------------
            for half in range(2):
                psS = psm.tile([H, 512], F32, tag="psm", name=f"psS{half}")
                for c in range(NK):
                    nc.tensor.matmul(
                        psS[:, :], lhsT=ugT[:, H * c : H * (c + 1)],
                        rhs=znT_all[:, D * c + 512 * half : D * c + 512 * (half + 1)],
                        start=(c == 0), stop=(c == NK - 1),
                    )
                nc.scalar.activation(
                    out=a_sb[:, 1 + 512 * half : 1 + 512 * (half + 1)], in_=psS[:, :],
                    func=AF.Exp, bias=eb[:, 0:1], scale=0.125,
                    accum_out=se[:, half : half + 1],
                )
                for kk in range(4):
                    k = 4 * half + kk
                    atpk = pt.tile([128, 16], BF16, tag="pt", name=f"atp{k}")
                    nc.tensor.transpose(
                        out=atpk[:, 0:H],
                        in_=a_sb[:, 1 + 128 * k : 1 + 128 * (k + 1)],
                        identity=ident[0:H, 0:H],
                    )
                    nc.scalar.copy(out=aT[:, H * k : H * (k + 1)], in_=atpk[:, 0:H])
                    for dh in range(2):
                        nc.tensor.matmul(
                            psM[:, 512 * dh : 512 * (dh + 1)], lhsT=aT[:, H * k : H * (k + 1)],
                            rhs=znorm_all[:, D * k + 512 * dh : D * k + 512 * (dh + 1)],
                            start=False, stop=(k == NK - 1), skip_group_check=True,
                        )

            rinv = work.tile([H, 1], F32, tag="rinv")
            nc.vector.reduce_sum(out=rinv[:, :], in_=se[:, :], axis=AX)
            nc.vector.reciprocal(out=rinv[:, :], in_=rinv[:, :])

            # mrow = psM * rinv  (gamma/beta fold applied after transpose)
            mrow = singles.tile([H, D], BF16, tag="mrow")
            nc.vector.tensor_scalar_mul(out=mrow[:, :], in0=psM[:, :], scalar1=rinv[:, 0:1])
            mT = singles.tile([128, H * NK], BF16, tag="mT")
            for c in range(NK):
                mtp = pt.tile([128, 16], BF16, tag="pt", name=f"mtp{c}")
                nc.tensor.transpose(
                    out=mtp[:, 0:H], in_=mrow[:, 128 * c : 128 * (c + 1)],
                    identity=ident[0:H, 0:H],
                )
                # fold PSUM->SBUF copy into the gamma*x+beta apply
                nc.vector.tensor_scalar(
                    out=mT[:, H * c : H * (c + 1)], in0=mtp[:, 0:H],
                    scalar1=gam_col[:, c : c + 1], scalar2=bet_col[:, c : c + 1],
                    op0=OP.mult, op1=OP.add,
                )

            # ---- ctx via [16,512] trick + transpose-select --------------
            # ps16[h, n] = sum_d Mln[h, d] * w_v[n, d]; ctx[n] = ps16[n//64, n]
            # c-outer so chunks 0-3 start when the first wv half lands
            ps16 = pbig.tile([H, D], F32, tag="pbig")
            for c in range(NK):
                for half in range(2):
                    nc.tensor.matmul(
                        ps16[:, 512 * half : 512 * (half + 1)], lhsT=mT[:, H * c : H * (c + 1)],
                        rhs=wv_all[:, D * c + 512 * half : D * c + 512 * (half + 1)],
                        start=(c == 0), stop=(c == NK - 1),
                        skip_group_check=True,
                    )
            c16 = singles.tile([H, D], BF16, tag="c16")
            nc.scalar.copy(out=c16[:, :], in_=ps16[:, :])
            ctx_bf = singles.tile([128, NK], BF16, tag="ctxbf")
            psO = pbig.tile([1, D], F32, tag="pbig")
            for j in range(NK):
                ctp = pt.tile([128, 16], BF16, tag="pt", name=f"ctp{j}")
                nc.tensor.transpose(
                    out=ctp[:, 0:H], in_=c16[:, 128 * j : 128 * (j + 1)],
                    identity=ident[0:H, 0:H],
                )
                nc.vector.scalar_tensor_tensor(
                    out=ctx_bf[0:64, j : j + 1], in0=ctp[0:64, 2 * j : 2 * j + 1],
                    scalar=1.0, in1=bv_col[0:64, j : j + 1], op0=OP.mult, op1=OP.add,
                )
                nc.vector.scalar_tensor_tensor(
                    out=ctx_bf[64:128, j : j + 1], in0=ctp[64:128, 2 * j + 1 : 2 * j + 2],
                    scalar=1.0, in1=bv_col[64:128, j : j + 1], op0=OP.mult, op1=OP.add,
                )
            for a in range(NK):
                for half in range(2):
                    nc.tensor.matmul(
                        psO[:, 512 * half : 512 * (half + 1)],
                        lhsT=ctx_bf[:, a : a + 1],
                        rhs=wo_all[:, D * a + 512 * half : D * a + 512 * (half + 1)],
                        start=(a == 0), stop=False,
                        skip_group_check=True,
                    )
            for half in range(2):
                nc.tensor.matmul(
                    psO[:, 512 * half : 512 * (half + 1)], lhsT=one_t[:, :],
                    rhs=bo_row[0:1, 512 * half : 512 * (half + 1)],
                    start=False, stop=True, skip_group_check=True,
                )
            out_sb = singles.tile([1, D], F32, tag="outsb")
            nc.scalar.copy(out=out_sb[:, :], in_=psO[:, :])
            nc.scalar.dma_start(out=out_e[:, :], in_=out_sb[:, :])

    nc.compile()
    return nc


def _pack128(a):
    rows, cols = a.shape
    return np.ascontiguousarray(
        a.reshape(NK, 128, cols).transpose(1, 0, 2).reshape(128, NK * cols)
    )


def _col(a):
    return np.ascontiguousarray(a.reshape(NK, 128).T)


def _prep_in_maps(inputs):
    bf = ml_dtypes.bfloat16
    f32 = np.float32

    def c(a, dt):
        return np.asarray(a, dtype=dt)

    x = c(inputs["x"], bf)
    smalls = np.concatenate(
        [
            _col(c(inputs["gamma"], f32)),
            _col(c(inputs["beta"], f32)),
            _col(c(inputs["b_q"], f32)),
            _col(c(inputs["b_v"], f32)),
        ],
        axis=1,
    )
    clsbo = np.stack([c(inputs["cls_token"], f32), c(inputs["b_o"], f32)])
    wkp = np.concatenate([_pack128(c(inputs["w_k"], bf)), _col(c(inputs["b_k"], bf))], axis=1)
    shared = {
        "gam": np.ascontiguousarray(c(inputs["gamma"], f32)),
        "bet": np.ascontiguousarray(c(inputs["beta"], f32)),
        "smalls": np.ascontiguousarray(smalls),
        "clsbo": np.ascontiguousarray(clsbo),
        "wqp": _pack128(c(np.asarray(inputs["w_q"]).T, bf)),
        "wkp": np.ascontiguousarray(wkp),
        "wvp": _pack128(c(np.asarray(inputs["w_v"]).T, bf)),
        "wop": _pack128(c(np.asarray(inputs["w_o"]).T, bf)),
    }
    return [{"x": _pack128(x[b]), **shared} for b in range(8)]


def run(inputs, trace=False, **kw):
    if "nc" not in _CACHE:
        _CACHE["nc"] = _build()
    nc = _CACHE["nc"]
    in_maps = _prep_in_maps(inputs)
    res = run_bass_kernel_spmd(nc, in_maps, core_ids=list(range(8)), trace=trace, **kw)
    out = np.stack([np.asarray(res.results[b]["out"][0], dtype=np.float32) for b in range(8)])
    return out, res


def kernel(**inputs):
    out, _ = run(inputs, trace=False)
    return out
